# revision 1
# baseline (speedup 1.0000x reference)
"""GAT 2-layer propagation kernel for Trainium2, 8 NeuronCores (SPMD).

Strategy (edge-parallel, dst-node-range sharded across 8 cores):
  - Core c owns dst nodes [c*6250, (c+1)*6250); edges (with self-loops) go to
    the core owning their dst, so each core emits its contiguous output rows
    and no reduction collective is needed.
  - Per layer a DRAM gather table holds one fp16 row per node:
      G1[n] = [h1 x128 | as1 x4 | ad1 x4 | pad]   (512 B rows)
      G2[n] = [h2 x32  | as2    | ad2    | pad]   (256 B rows)
    (h carries the layer bias folded in: softmax weights sum to 1, so adding
    b to every value row adds b to the output.)
  - Per 128-dst tile, edges live in "chunks" of 128 slots (partition dim).
    Aligned chunks put dst p's k-th edge at partition p: the segment-sum
    matmul then uses a constant identity stationary operand and alpha_dst is
    a per-partition tile fetched once per tile. Overflow edges (degree > K0)
    go to a few dynamic chunks with is_equal(iota, dstloc) masks; their
    alpha_dst comes from a PE-transposed-mask matmul against the ad tile.
  - Value rows are fetched with dma_gather (int16 indices): two calls per
    tile over table halves [0, 32768) and [32768, N) since src ids exceed
    int16 range; chunks are src-homogeneous by construction. Pad slots gather
    row 0 and a host validity mask zeroes their attention weight.
  - e = exp(leakyrelu(as+ad)) needs no max-subtraction (|alpha| <= ~6 here,
    far from fp32 exp overflow). Aggregation accumulates [sum e*h | sum e]
    in fp32 PSUM; the softmax division happens once per dst at the end.
  - Between layers each core builds table rows for its own nodes; an
    AllGather + relayout replicates the table. The final sigmoid runs as one
    deferred sweep so ACT's activation table stays on Exp during edge phases.
"""

import numpy as np

import concourse.bacc as bacc
import concourse.tile as tile
from concourse import mybir
from concourse.bass import IndirectOffsetOnAxis
from concourse.bass_utils import run_bass_kernel_spmd

F32 = mybir.dt.float32
F16 = mybir.dt.float16
I32 = mybir.dt.int32
I16 = mybir.dt.int16
AF = mybir.ActivationFunctionType
OP = mybir.AluOpType

P = 128
HALF = 32768            # int16-addressable rows per dma_gather call


class GATConfig:
    def __init__(self, n, e, in_dim=128, hid=32, heads=4, out_dim=32,
                 neg_slope=0.2, n_cores=8):
        assert in_dim == P and heads * hid == P
        self.N, self.E = n, e
        self.HID = hid
        self.H = heads
        self.OUT = out_dim
        self.NEG = neg_slope
        self.NC = n_cores
        assert n % n_cores == 0
        self.NPC = n // n_cores
        self.NT = (self.NPC + P - 1) // P
        self.LAST = self.NPC - (self.NT - 1) * P
        self.C1 = heads * hid                 # 128
        self.G1W = 256                        # fp16 els/row: h|as|ad|pad
        self.G2W = 128
        self.NNT = (n + P - 1) // P
        self.LASTN = n - (self.NNT - 1) * P


class EdgePlan:
    """Chunk structure picked from the actual degree distribution."""


def _pick_k0(deg):
    """deg: [NC, NT, P] int. Returns (K0, OV) minimizing chunk count."""
    best = None
    hi = int(deg.max())
    for k0 in range(0, hi + 1):
        ovf = np.maximum(deg - k0, 0).sum(axis=2)          # [NC, NT]
        ov = int(np.ceil(ovf / P).max()) if ovf.max() > 0 else 0
        cost = k0 + 1.5 * ov
        if best is None or cost < best[0]:
            best = (cost, k0, ov)
    return best[1], best[2]


def _prep_host(cfg, x, edge_index, W1, a_src1, a_dst1, b1, W2, a_src2,
               a_dst2, b2):
    N, H, HID = cfg.N, cfg.H, cfg.HID
    NPC, NT, NC = cfg.NPC, cfg.NT, cfg.NC

    src = np.concatenate([np.asarray(edge_index[0], dtype=np.int64),
                          np.arange(N, dtype=np.int64)])
    dst = np.concatenate([np.asarray(edge_index[1], dtype=np.int64),
                          np.arange(N, dtype=np.int64)])
    order = np.argsort(dst, kind="stable")
    src, dst = src[order], dst[order]

    core_of = dst // NPC
    tile_of = (dst % NPC) // P
    part_of = (dst % NPC) % P
    is_hi = src >= HALF

    deg_lo = np.zeros((NC, NT, P), np.int64)
    deg_hi = np.zeros((NC, NT, P), np.int64)
    np.add.at(deg_lo, (core_of[~is_hi], tile_of[~is_hi], part_of[~is_hi]), 1)
    np.add.at(deg_hi, (core_of[is_hi], tile_of[is_hi], part_of[is_hi]), 1)

    K0L, OVL = _pick_k0(deg_lo)
    K0H, OVH = _pick_k0(deg_hi)
    NOV = OVL + OVH
    S = OVL + K0L + K0H + OVH
    plan = EdgePlan()
    plan.K0L, plan.OVL, plan.K0H, plan.OVH, plan.S = K0L, OVL, K0H, OVH, S

    CL, CH = OVL + K0L, K0H + OVH          # chunks per lo/hi gather call

    def pack16(vals):
        # vals: [n_chunks*P] int16 in slot order j=k*128+p; idx j lives at
        # [j%16, j//16], replicated across the 8 stripes of 16 partitions.
        a = vals.reshape(-1, 16).T.astype(np.int16)   # [16, n/16]
        return np.tile(a, (8, 1))

    per_core = []
    for c in range(NC):
        m = core_of == c
        s_c = src[m].astype(np.int64)
        t_c = tile_of[m]
        p_c = part_of[m]
        ilo = np.zeros((NT, max(CL, 1) * P), np.int16)
        ihi = np.zeros((NT, max(CH, 1) * P), np.int16)
        vm = np.zeros((NT, S, P), np.float16)
        dlov = np.full((NT, max(NOV, 1), P), 1000.0, np.float32)
        for t in range(NT):
            mt = t_c == t
            s_t, p_t = s_c[mt], p_c[mt]
            lo_t = s_t < HALF
            ov_lo, ov_hi = [], []
            for p in range(P):
                es = s_t[(p_t == p) & lo_t]
                nk = min(len(es), K0L)
                for k in range(nk):
                    ilo[t, (OVL + k) * P + p] = es[k]
                vm[t, OVL:OVL + nk, p] = 1.0
                ov_lo.extend((s, p) for s in es[K0L:])
                es = s_t[(p_t == p) & ~lo_t]
                nk = min(len(es), K0H)
                for k in range(nk):
                    ihi[t, k * P + p] = es[k] - HALF
                vm[t, OVL + K0L:OVL + K0L + nk, p] = 1.0
                ov_hi.extend((s, p) for s in es[K0H:])
            assert len(ov_lo) <= OVL * P and len(ov_hi) <= OVH * P
            for j, (s, p) in enumerate(ov_lo):
                ilo[t, (j // P) * P + j % P] = s
                vm[t, j // P, j % P] = 1.0
                dlov[t, j // P, j % P] = float(p)
            for j, (s, p) in enumerate(ov_hi):
                ihi[t, (K0H + j // P) * P + j % P] = s - HALF
                vm[t, S - OVH + j // P, j % P] = 1.0
                dlov[t, OVL + j // P, j % P] = float(p)

        plo = np.concatenate([pack16(ilo[t]) for t in range(NT)], axis=1) \
            if CL else np.zeros((P, NT * 8), np.int16)
        phi = np.concatenate([pack16(ihi[t]) for t in range(NT)], axis=1) \
            if CH else np.zeros((P, NT * 8), np.int16)
        adrows = (c * NPC + np.arange(NT)[None, :] * P
                  + np.arange(P)[:, None]).astype(np.int32)
        np.clip(adrows, 0, N + P - 1, out=adrows)
        per_core.append({
            "idxlo": np.ascontiguousarray(plo),
            "idxhi": np.ascontiguousarray(phi),
            "vmask": np.ascontiguousarray(
                vm.transpose(2, 0, 1).reshape(P, NT * S)),
            "dlocov": np.ascontiguousarray(
                dlov.transpose(2, 0, 1).reshape(P, NT * max(NOV, 1))),
            "adrows": np.ascontiguousarray(adrows),
        })

    # block-diagonal attention projectors: as1 = h1 @ asrc_blk
    asrc_blk = np.zeros((cfg.C1, H), np.float32)
    adst_blk = np.zeros((cfg.C1, H), np.float32)
    for h in range(H):
        asrc_blk[h * HID:(h + 1) * HID, h] = a_src1[h]
        adst_blk[h * HID:(h + 1) * HID, h] = a_dst1[h]

    b1row = np.zeros((1, cfg.C1 + 2 * H), np.float32)
    b1row[0, :cfg.C1] = b1
    b2row = np.zeros((1, cfg.OUT + 2), np.float16)
    b2row[0, :cfg.OUT] = b2

    common = {
        "xT": np.ascontiguousarray(np.asarray(x, np.float32).T),
        "W1": np.ascontiguousarray(np.asarray(W1, np.float32)),
        "W1T": np.ascontiguousarray(np.asarray(W1, np.float32).T),
        "asrcblk": asrc_blk, "adstblk": adst_blk, "b1row": b1row,
        "W2": np.ascontiguousarray(np.asarray(W2, np.float32)),
        "W2T": np.ascontiguousarray(np.asarray(W2, np.float32).T),
        "a2src": np.ascontiguousarray(
            np.asarray(a_src2, np.float32).reshape(-1, 1)),
        "a2dst": np.ascontiguousarray(
            np.asarray(a_dst2, np.float32).reshape(-1, 1)),
        "b2row": b2row,
        "iotah": np.ascontiguousarray(
            np.tile(np.arange(P, dtype=np.float16), (P, 1))),
        "identh": np.eye(P, dtype=np.float16),
        "onesrow": np.ones((1, P), np.float32),
        "onesrowh": np.ones((1, P), np.float16),
    }
    return plan, common, per_core


def _build(cfg, plan):
    N, H, HID, C1 = cfg.N, cfg.H, cfg.HID, cfg.C1
    NT, NPC, NNT = cfg.NT, cfg.NPC, cfg.NNT
    C2 = cfg.OUT
    K0L, OVL, K0H, OVH, S = plan.K0L, plan.OVL, plan.K0H, plan.OVH, plan.S
    NOV = OVL + OVH
    CL, CH = OVL + K0L, K0H + OVH
    G1W, G2W = cfg.G1W, cfg.G2W
    G1C = C1 + 2 * H                       # 136 used cols in phase A psum
    G2C = C2 + 2                           # 34 used cols in phase C psum
    AL0, AL1 = OVL, OVL + K0L + K0H        # contiguous aligned chunk range
    ov_ids = list(range(OVL)) + list(range(S - OVH, S))

    nc = bacc.Bacc("TRN2", target_bir_lowering=False, debug=False,
                   num_devices=cfg.NC)

    def din(name, shape, dt=F32):
        return nc.dram_tensor(name, shape, dt, kind="ExternalInput").ap()

    xT = din("xT", [P, N])
    W1 = din("W1", [P, C1])
    W1T = din("W1T", [C1, P])
    asrcblk = din("asrcblk", [C1, H])
    adstblk = din("adstblk", [C1, H])
    b1row = din("b1row", [1, G1C])
    W2 = din("W2", [C1, C2])
    W2T = din("W2T", [C2, C1])
    a2src = din("a2src", [C2, 1])
    a2dst = din("a2dst", [C2, 1])
    b2row = din("b2row", [1, G2C], F16)
    iotah = din("iotah", [P, P], F16)
    identh = din("identh", [P, P], F16)
    onesrow = din("onesrow", [1, P])
    onesrowh = din("onesrowh", [1, P], F16)
    idxlo = din("idxlo", [P, NT * max(CL, 1) * 8], I16)
    idxhi = din("idxhi", [P, NT * max(CH, 1) * 8], I16)
    vmask = din("vmask", [P, NT * S], F16)
    dlocov = din("dlocov", [P, NT * max(NOV, 1)])
    adrows = din("adrows", [P, NT], I32)

    out = nc.dram_tensor("out", [NT * P, C2], F32, kind="ExternalOutput").ap()

    G1 = nc.dram_tensor("G1", [N + P, G1W], F16).ap()
    G2 = nc.dram_tensor("G2", [N + P, G2W], F16).ap()
    G2c = nc.dram_tensor("G2c", [NPC, G2C], F16).ap()
    G2cf = nc.dram_tensor("G2cf", [N, G2C], F16, addr_space="Shared").ap()
    PRE = nc.dram_tensor("PRE", [NT * P, C2], F32).ap()

    with tile.TileContext(nc) as tc:
        with (
            tc.tile_pool(name="const", bufs=1) as const,
            tc.tile_pool(name="cpsum", bufs=1, space="PSUM") as cpsum,
            tc.tile_pool(name="o1p", bufs=NT) as o1p,
        ):
            # ---- constants / fused weight tables --------------------------
            w1ext = const.tile([P, G1C], F32)       # [W1 | W1@Asrc | W1@Adst]
            nc.sync.dma_start(out=w1ext[:, 0:C1], in_=W1)
            w1t = const.tile([P, P], F32)
            nc.sync.dma_start(out=w1t[:], in_=W1T)
            ablk = const.tile([P, 2 * H], F32)
            nc.sync.dma_start(out=ablk[:, 0:H], in_=asrcblk)
            nc.sync.dma_start(out=ablk[:, H:2 * H], in_=adstblk)
            pw = cpsum.tile([P, 2 * H], F32, space="PSUM")
            nc.tensor.matmul(pw[:], lhsT=w1t[:], rhs=ablk[:], start=True,
                             stop=True)
            nc.vector.tensor_copy(out=w1ext[:, C1:C1 + 2 * H], in_=pw[:])

            w2ext = const.tile([P, G2C], F16)       # [W2 | W2@a2s | W2@a2d]
            nc.gpsimd.dma_start(out=w2ext[:, 0:C2], in_=W2)   # f32 -> f16
            w2t = const.tile([C2, C1], F32)
            nc.sync.dma_start(out=w2t[:], in_=W2T)
            a2 = const.tile([C2, 2], F32)
            nc.sync.dma_start(out=a2[:, 0:1], in_=a2src)
            nc.sync.dma_start(out=a2[:, 1:2], in_=a2dst)
            pw2 = cpsum.tile([P, 2], F32, space="PSUM")
            nc.tensor.matmul(pw2[:], lhsT=w2t[:], rhs=a2[:], start=True,
                             stop=True)
            nc.vector.tensor_copy(out=w2ext[:, C2:C2 + 2], in_=pw2[:])

            b1sb = const.tile([1, G1C], F32)
            nc.sync.dma_start(out=b1sb[:], in_=b1row)
            b2sb = const.tile([1, G2C], F16)
            nc.sync.dma_start(out=b2sb[:], in_=b2row)
            iosb = const.tile([P, P], F16)
            nc.sync.dma_start(out=iosb[:], in_=iotah)
            idsb = const.tile([P, P], F16)
            nc.sync.dma_start(out=idsb[:], in_=identh)
            onesb = const.tile([1, P], F32)
            nc.sync.dma_start(out=onesb[:], in_=onesrow)
            onehb = const.tile([1, P], F16)
            nc.sync.dma_start(out=onehb[:], in_=onesrowh)
            adr = const.tile([P, NT], I32)
            nc.sync.dma_start(out=adr[:], in_=adrows)

            # ---- phase A: G1 rows -----------------------------------------
            with (
                tc.tile_pool(name="pa", bufs=3) as pa,
                tc.tile_pool(name="pap", bufs=3, space="PSUM") as pap,
            ):
                zz = pa.tile([P, G1W], F16, tag="zz")
                nc.vector.memset(zz[:], 0.0)
                nc.sync.dma_start(out=G1[N:N + P, :], in_=zz[:])
                nc.sync.dma_start(out=G2[N:N + P, :], in_=zz[:, 0:G2W])
                for i in range(NNT):
                    nn = P if i < NNT - 1 else cfg.LASTN
                    xt = pa.tile([P, P], F32, tag="xt")
                    nc.sync.dma_start(out=xt[:, :nn],
                                      in_=xT[:, i * P:i * P + nn])
                    ps = pap.tile([P, G1C], F32, space="PSUM", tag="ps")
                    nc.tensor.matmul(ps[:nn, :], lhsT=xt[:, :nn], rhs=w1ext[:],
                                     start=True, stop=False)
                    nc.tensor.matmul(ps[:nn, :], lhsT=onesb[:1, :nn],
                                     rhs=b1sb[:], start=False, stop=True)
                    g1h = pa.tile([P, G1C], F16, tag="g1h")
                    nc.scalar.copy(out=g1h[:nn, :], in_=ps[:nn, :])
                    nc.sync.dma_start(out=G1[i * P:i * P + nn, 0:G1C],
                                      in_=g1h[:nn, :])

            # ---- phase B: layer-1 edge aggregation ------------------------
            o1_tiles = []
            with (
                tc.tile_pool(name="pbig", bufs=2) as pbig,
                tc.tile_pool(name="pmed", bufs=2) as pmed,
                tc.tile_pool(name="pmsk", bufs=2 * max(NOV, 1) + 2) as pmsk,
                tc.tile_pool(name="pbp", bufs=2, space="PSUM") as pbp,
                tc.tile_pool(name="pbpt", bufs=2, space="PSUM") as pbpt,
                tc.tile_pool(name="pbpa", bufs=2, space="PSUM") as pbpa,
            ):
                for t in range(NT):
                    ndst = P if t < NT - 1 else cfg.LAST
                    vg = pbig.tile([P, S * G1W], F16, tag="vg")
                    vg3 = vg[:].rearrange("p (k c) -> p k c", c=G1W)
                    if CL:
                        ilo = pmed.tile([P, CL * 8], I16, tag="ilo")
                        nc.sync.dma_start(
                            out=ilo[:],
                            in_=idxlo[:, t * CL * 8:(t + 1) * CL * 8])
                        nc.gpsimd.dma_gather(
                            out_ap=vg3[:, 0:CL, :],
                            in_ap=G1[0:min(HALF, N + P), :],
                            idxs_ap=ilo[:], num_idxs=CL * P,
                            num_idxs_reg=CL * P, elem_size=G1W, single_packet=False)
                    if CH:
                        ihi = pmed.tile([P, CH * 8], I16, tag="ihi")
                        nc.sync.dma_start(
                            out=ihi[:],
                            in_=idxhi[:, t * CH * 8:(t + 1) * CH * 8])
                        nc.gpsimd.dma_gather(
                            out_ap=vg3[:, CL:S, :], in_ap=G1[HALF:N + P, :],
                            idxs_ap=ihi[:], num_idxs=CH * P,
                            num_idxs_reg=CH * P, elem_size=G1W, single_packet=False)
                    vm = pmed.tile([P, S], F16, tag="vm")
                    nc.sync.dma_start(out=vm[:],
                                      in_=vmask[:, t * S:(t + 1) * S])
                    # alpha_dst values of this tile's dsts, per partition
                    adt = pmed.tile([P, H], F16, tag="adt")
                    nc.gpsimd.indirect_dma_start(
                        out=adt[:], out_offset=None, in_=G1,
                        in_offset=IndirectOffsetOnAxis(ap=adr[:, t:t + 1],
                                                       axis=0),
                        element_offset=C1 + H)

                    alp = pmed.tile([P, S * H], F32, tag="alp")
                    alp3 = alp[:].rearrange("p (k h) -> p k h", h=H)
                    nc.vector.tensor_tensor(
                        out=alp3[:, AL0:AL1, :],
                        in0=vg3[:, AL0:AL1, C1:C1 + H],
                        in1=adt[:].rearrange("p (o h) -> p o h", o=1)
                            .to_broadcast([P, AL1 - AL0, H]),
                        op=OP.add)
                    msks = {}
                    dlo = pmed.tile([P, max(NOV, 1)], F32, tag="dlo")
                    if NOV:
                        nc.sync.dma_start(
                            out=dlo[:],
                            in_=dlocov[:, t * NOV:(t + 1) * NOV])
                    for jj, k in enumerate(ov_ids):
                        msk = pmsk.tile([P, P], F16, tag="msk")
                        nc.vector.tensor_scalar(
                            out=msk[:], in0=iosb[:],
                            scalar1=dlo[:, jj:jj + 1], scalar2=None,
                            op0=OP.is_equal)
                        msks[k] = msk
                        tp = pbpt.tile([P, P], F16, space="PSUM", tag="tp")
                        nc.tensor.transpose(out=tp[:], in_=msk[:],
                                            identity=idsb[:])
                        mskT = pmsk.tile([P, P], F16, tag="mskT")
                        nc.vector.tensor_copy(out=mskT[:], in_=tp[:])
                        adp = pbpa.tile([P, H], F32, space="PSUM", tag="adp")
                        nc.tensor.matmul(adp[:], lhsT=mskT[:], rhs=adt[:],
                                         start=True, stop=True)
                        nc.vector.tensor_tensor(
                            out=alp3[:, k, :], in0=vg3[:, k, C1:C1 + H],
                            in1=adp[:], op=OP.add)
                    # e = exp(lrelu(alpha)) * vmask
                    asc = pmed.tile([P, S * H], F32, tag="asc")
                    nc.vector.tensor_scalar(out=asc[:], in0=alp[:],
                                            scalar1=cfg.NEG, scalar2=None,
                                            op0=OP.mult)
                    lrl = pmed.tile([P, S * H], F32, tag="lrl")
                    nc.vector.tensor_tensor(out=lrl[:], in0=alp[:], in1=asc[:],
                                            op=OP.max)
                    ee = pmed.tile([P, S * H], F32, tag="ee")
                    nc.scalar.activation(out=ee[:], in_=lrl[:], func=AF.Exp)
                    eeh = pmed.tile([P, S * H], F16, tag="eeh")
                    nc.vector.tensor_tensor(
                        out=eeh[:].rearrange("p (k h) -> p k h", h=H),
                        in0=ee[:].rearrange("p (k h) -> p k h", h=H),
                        in1=vm[:].rearrange("p (k o) -> p k o", o=1)
                            .to_broadcast([P, S, H]),
                        op=OP.mult)
                    eeh3 = eeh[:].rearrange("p (k h) -> p k h", h=H)
                    # rhs = [e*h | e]
                    xx = pbig.tile([P, S * (C1 + H)], F16, tag="xx")
                    xx3 = xx[:].rearrange("p (k c) -> p k c", c=C1 + H)
                    nc.vector.tensor_copy(out=xx3[:, :, C1:C1 + H], in_=eeh3)
                    nc.vector.tensor_tensor(
                        out=xx3[:, :, 0:C1].rearrange("p k (h c) -> p k h c",
                                                      c=HID),
                        in0=vg3[:, :, 0:C1].rearrange("p k (h c) -> p k h c",
                                                      c=HID),
                        in1=eeh[:].rearrange("p (k h o) -> p k h o", h=H, o=1)
                            .to_broadcast([P, S, H, HID]),
                        op=OP.mult)
                    ps = pbp.tile([P, C1 + H], F32, space="PSUM", tag="ps")
                    for k in range(S):
                        lhsT = msks[k] if k in msks else idsb
                        nc.tensor.matmul(ps[:], lhsT=lhsT[:],
                                         rhs=xx3[:, k, :],
                                         start=(k == 0), stop=(k == S - 1))
                    rec = pmed.tile([P, H], F32, tag="rec")
                    nc.vector.reciprocal(out=rec[:ndst, :],
                                         in_=ps[:ndst, C1:C1 + H])
                    o1 = o1p.tile([P, C1], F16, tag="o1")
                    if ndst < P:
                        nc.vector.memset(o1[:], 0.0)
                    for h in range(H):
                        nc.vector.tensor_scalar(
                            out=o1[:ndst, h * HID:(h + 1) * HID],
                            in0=ps[:ndst, h * HID:(h + 1) * HID],
                            scalar1=rec[:ndst, h:h + 1], scalar2=0.0,
                            op0=OP.mult, op1=OP.max)
                    o1_tiles.append(o1)

            # ---- phase C: G2 rows + allgather -----------------------------
            with (
                tc.tile_pool(name="pc", bufs=3) as pc,
                tc.tile_pool(name="pcp", bufs=2, space="PSUM") as pcp,
                tc.tile_pool(name="pcpt", bufs=2, space="PSUM") as pcpt,
            ):
                for t in range(NT):
                    ndst = P if t < NT - 1 else cfg.LAST
                    tp = pcpt.tile([P, P], F16, space="PSUM", tag="tp")
                    nc.tensor.transpose(out=tp[:], in_=o1_tiles[t][:],
                                        identity=idsb[:])
                    o1t = pc.tile([P, P], F16, tag="o1t")
                    nc.vector.tensor_copy(out=o1t[:], in_=tp[:])
                    hp = pcp.tile([P, G2C], F32, space="PSUM", tag="hp")
                    nc.tensor.matmul(hp[:], lhsT=o1t[:], rhs=w2ext[:],
                                     start=True, stop=False)
                    nc.tensor.matmul(hp[:], lhsT=onehb[:1, :], rhs=b2sb[:],
                                     start=False, stop=True)
                    g2h = pc.tile([P, G2C], F16, tag="g2h")
                    nc.vector.tensor_copy(out=g2h[:ndst, :], in_=hp[:ndst, :])
                    nc.sync.dma_start(out=G2c[t * P:t * P + ndst, :],
                                      in_=g2h[:ndst, :])

            nc.gpsimd.collective_compute(
                "AllGather", OP.bypass,
                replica_groups=[list(range(cfg.NC))],
                ins=[G2c], outs=[G2cf])
            nc.sync.dma_start(out=G2[0:N, 0:G2C], in_=G2cf[:, :])

            # ---- phase D: layer-2 edge aggregation (1 head) ---------------
            with (
                tc.tile_pool(name="pdig", bufs=2) as pdig,
                tc.tile_pool(name="pdmd", bufs=2) as pdmd,
                tc.tile_pool(name="pdmk", bufs=2 * max(NOV, 1) + 2) as pdmk,
                tc.tile_pool(name="pdp", bufs=2, space="PSUM") as pdp,
                tc.tile_pool(name="pdpt", bufs=2, space="PSUM") as pdpt,
                tc.tile_pool(name="pdpa", bufs=2, space="PSUM") as pdpa,
            ):
                for t in range(NT):
                    ndst = P if t < NT - 1 else cfg.LAST
                    vg = pdig.tile([P, S * G2W], F16, tag="vg2")
                    vg3 = vg[:].rearrange("p (k c) -> p k c", c=G2W)
                    if CL:
                        ilo = pdmd.tile([P, CL * 8], I16, tag="ilo")
                        nc.sync.dma_start(
                            out=ilo[:],
                            in_=idxlo[:, t * CL * 8:(t + 1) * CL * 8])
                        nc.gpsimd.dma_gather(
                            out_ap=vg3[:, 0:CL, :],
                            in_ap=G2[0:min(HALF, N + P), :],
                            idxs_ap=ilo[:], num_idxs=CL * P,
                            num_idxs_reg=CL * P, elem_size=G2W, single_packet=False)
                    if CH:
                        ihi = pdmd.tile([P, CH * 8], I16, tag="ihi")
                        nc.sync.dma_start(
                            out=ihi[:],
                            in_=idxhi[:, t * CH * 8:(t + 1) * CH * 8])
                        nc.gpsimd.dma_gather(
                            out_ap=vg3[:, CL:S, :], in_ap=G2[HALF:N + P, :],
                            idxs_ap=ihi[:], num_idxs=CH * P,
                            num_idxs_reg=CH * P, elem_size=G2W, single_packet=False)
                    vm = pdmd.tile([P, S], F16, tag="vm")
                    nc.sync.dma_start(out=vm[:],
                                      in_=vmask[:, t * S:(t + 1) * S])
                    adt = pdmd.tile([P, 1], F16, tag="adt2")
                    nc.gpsimd.indirect_dma_start(
                        out=adt[:], out_offset=None, in_=G2,
                        in_offset=IndirectOffsetOnAxis(ap=adr[:, t:t + 1],
                                                       axis=0),
                        element_offset=C2 + 1)

                    alp = pdmd.tile([P, S], F32, tag="alp2")
                    alp3 = alp[:].rearrange("p (k o) -> p k o", o=1)
                    nc.vector.tensor_tensor(
                        out=alp3[:, AL0:AL1, :],
                        in0=vg3[:, AL0:AL1, C2:C2 + 1],
                        in1=adt[:].rearrange("p (o h) -> p o h", o=1)
                            .to_broadcast([P, AL1 - AL0, 1]),
                        op=OP.add)
                    msks = {}
                    dlo = pdmd.tile([P, max(NOV, 1)], F32, tag="dlo2")
                    if NOV:
                        nc.sync.dma_start(
                            out=dlo[:],
                            in_=dlocov[:, t * NOV:(t + 1) * NOV])
                    for jj, k in enumerate(ov_ids):
                        msk = pdmk.tile([P, P], F16, tag="msk2")
                        nc.vector.tensor_scalar(
                            out=msk[:], in0=iosb[:],
                            scalar1=dlo[:, jj:jj + 1], scalar2=None,
                            op0=OP.is_equal)
                        msks[k] = msk
                        tp = pdpt.tile([P, P], F16, space="PSUM", tag="tp2")
                        nc.tensor.transpose(out=tp[:], in_=msk[:],
                                            identity=idsb[:])
                        mskT = pdmk.tile([P, P], F16, tag="mskT2")
                        nc.vector.tensor_copy(out=mskT[:], in_=tp[:])
                        adp = pdpa.tile([P, 1], F32, space="PSUM", tag="adp2")
                        nc.tensor.matmul(adp[:], lhsT=mskT[:], rhs=adt[:],
                                         start=True, stop=True)
                        nc.vector.tensor_tensor(
                            out=alp3[:, k, :], in0=vg3[:, k, C2:C2 + 1],
                            in1=adp[:], op=OP.add)
                    asc = pdmd.tile([P, S], F32, tag="asc2")
                    nc.vector.tensor_scalar(out=asc[:], in0=alp[:],
                                            scalar1=cfg.NEG, scalar2=None,
                                            op0=OP.mult)
                    lrl = pdmd.tile([P, S], F32, tag="lrl2")
                    nc.vector.tensor_tensor(out=lrl[:], in0=alp[:], in1=asc[:],
                                            op=OP.max)
                    ee = pdmd.tile([P, S], F32, tag="ee2")
                    nc.scalar.activation(out=ee[:], in_=lrl[:], func=AF.Exp)
                    eeh = pdmd.tile([P, S], F16, tag="eeh2")
                    nc.vector.tensor_tensor(out=eeh[:], in0=ee[:], in1=vm[:],
                                            op=OP.mult)
                    xx = pdig.tile([P, S * (C2 + 1)], F16, tag="xx2")
                    xx3 = xx[:].rearrange("p (k c) -> p k c", c=C2 + 1)
                    nc.vector.tensor_copy(
                        out=xx3[:, :, C2:C2 + 1],
                        in_=eeh[:].rearrange("p (k o) -> p k o", o=1))
                    nc.vector.tensor_tensor(
                        out=xx3[:, :, 0:C2],
                        in0=vg3[:, :, 0:C2],
                        in1=eeh[:].rearrange("p (k o) -> p k o", o=1)
                            .to_broadcast([P, S, C2]),
                        op=OP.mult)
                    ps = pdp.tile([P, C2 + 1], F32, space="PSUM", tag="ps2")
                    for k in range(S):
                        lhsT = msks[k] if k in msks else idsb
                        nc.tensor.matmul(ps[:], lhsT=lhsT[:],
                                         rhs=xx3[:, k, :],
                                         start=(k == 0), stop=(k == S - 1))
                    rec = pdmd.tile([P, 1], F32, tag="rec2")
                    nc.vector.reciprocal(out=rec[:ndst, :],
                                         in_=ps[:ndst, C2:C2 + 1])
                    o2 = pdmd.tile([P, C2], F32, tag="o2")
                    nc.vector.tensor_scalar(out=o2[:ndst, :],
                                            in0=ps[:ndst, 0:C2],
                                            scalar1=rec[:ndst, :],
                                            scalar2=None, op0=OP.mult)
                    nc.sync.dma_start(out=PRE[t * P:t * P + ndst, :],
                                      in_=o2[:ndst, :])

            # ---- phase E: one sigmoid sweep -------------------------------
            with tc.tile_pool(name="pe", bufs=2) as pe:
                FW = NT * C2
                pre_f = PRE.rearrange("(a b) c -> a (b c)", a=P)
                out_f = out.rearrange("(a b) c -> a (b c)", a=P)
                pei = pe.tile([P, FW], F32, tag="pei")
                nc.sync.dma_start(out=pei[:], in_=pre_f)
                peo = pe.tile([P, FW], F32, tag="peo")
                nc.scalar.activation(out=peo[:], in_=pei[:], func=AF.Sigmoid)
                nc.sync.dma_start(out=out_f, in_=peo[:])

    nc.compile()
    return nc


_CACHE: dict = {}


def _get_module(cfg, plan):
    key = (cfg.N, cfg.E, plan.K0L, plan.OVL, plan.K0H, plan.OVH)
    if key not in _CACHE:
        _CACHE[key] = _build(cfg, plan)
    return _CACHE[key]


def _run(cfg, inputs, trace=False):
    plan, common, per_core = _prep_host(
        cfg, inputs["x"], inputs["edge_index"], inputs["W1"],
        inputs["a_src1"], inputs["a_dst1"], inputs["b1"], inputs["W2"],
        inputs["a_src2"], inputs["a_dst2"], inputs["b2"])
    nc = _get_module(cfg, plan)
    in_maps = [dict(common, **pc) for pc in per_core]
    res = run_bass_kernel_spmd(nc, in_maps, core_ids=list(range(cfg.NC)),
                               trace=trace)
    shards = [np.asarray(res.results[c]["out"])[:cfg.NPC]
              for c in range(cfg.NC)]
    full = np.concatenate(shards, axis=0).astype(np.float32)
    return (full, res) if trace else full


def kernel(**inputs) -> np.ndarray:
    cfg = GATConfig(n=50000, e=800000)
    return _run(cfg, inputs)



# revision 7
# speedup vs baseline: 1.1727x; 1.1727x over previous
"""GAT 2-layer propagation kernel for Trainium2, 8 NeuronCores (SPMD).

Strategy (edge-parallel, dst-node-range sharded across 8 cores):
  - Core c owns dst nodes [c*6250, (c+1)*6250); edges go to the core owning
    their dst, so each core emits its contiguous output rows; one AllGather
    per layer replicates the next layer's node table.
  - Per layer a DRAM table holds one row per node plus two PAD rows:
      G1 = [padlo | rows [h x128 | as x4 | ad x4 | junk] | padhi]  512 B/row
      G2 = [padlo | rows [h2 x32 | as2 | ad2 | junk]     | padhi]  256 B/row
    Pad rows carry as = -30000 so padded gather slots produce e = 0 after
    exp(leakyrelu(.)) — no separate validity mask or multiply. h carries the
    layer bias folded in (softmax weights sum to 1).
  - Per 128-dst tile, edges live in "chunks" of 128 slots (partition dim)
    with PER-TILE chunk counts (shapes shared across cores = max over the 8
    cores, so one BIR serves all SPMD cores; slot padding points at the pad
    rows). Aligned chunks put dst p's k-th edge at partition p: the
    segment-sum matmul uses an identity stationary operand and alpha_dst
    broadcasts from the tile's self-row columns. Overflow edges (degree >
    K0) go to densely packed chunks whose one-hot masks (and transposes,
    for the alpha_dst matmul) are built on the HOST and DMA'd in.
  - Self-loop edges are NOT gathered: layer 1 fetches each tile's own rows
    with one small indirect DMA (per-core row offsets are runtime inputs),
    which also provides as/ad columns for the aligned broadcast; layer 2
    keeps its own rows SBUF-resident from the inline layer-2 row phase.
  - Value rows are fetched with dma_gather (int16 indices) in two calls per
    tile over the table halves [0, 32768) and [32768, N+2); the SAME index
    and mask tables drive both layers. dma_gather descriptor generation on
    GpSimd (~8 ns/slot) is the critical resource; everything else overlaps
    under it.
  - e = exp(leakyrelu(as+ad)) needs no max-subtraction (|alpha| is far from
    fp32 exp overflow). Aggregation accumulates [sum e*h | sum e] in fp32
    PSUM; the softmax division happens once per dst. Layer-2 node rows are
    computed inline per tile (padded to 256 B so the AllGather output is
    gatherable directly after one dense copy), and the final sigmoid runs
    as one deferred sweep.
"""

import numpy as np

import concourse.bacc as bacc
import concourse.tile as tile
from concourse import mybir
from concourse.bass import IndirectOffsetOnAxis
from concourse.bass_utils import run_bass_kernel_spmd

F32 = mybir.dt.float32
F16 = mybir.dt.float16
I32 = mybir.dt.int32
I16 = mybir.dt.int16
AF = mybir.ActivationFunctionType
OP = mybir.AluOpType

P = 128
LO_MAX = 32767          # lo half rows 0..32767; node n lo iff n+1 <= 32767
NEG_AS = -30000.0       # pad-row alpha_src; exp(lrelu(NEG_AS+ad)) == 0 in f32


class GATConfig:
    def __init__(self, n, e, in_dim=128, hid=32, heads=4, out_dim=32,
                 neg_slope=0.2, n_cores=8):
        assert in_dim == P and heads * hid == P
        self.N, self.E = n, e
        self.HID = hid
        self.H = heads
        self.OUT = out_dim
        self.NEG = neg_slope
        self.NC = n_cores
        assert n % n_cores == 0
        self.NPC = n // n_cores
        self.NT = (self.NPC + P - 1) // P
        self.LAST = self.NPC - (self.NT - 1) * P
        self.C1 = heads * hid                 # 128
        self.G1W = 256                        # f16 els/row (512 B)
        self.G2W = 128                        # f16 els/row (256 B)
        self.G1C = self.C1 + 2 * heads        # 136 used cols
        self.G2C = self.OUT + 2               # 34 used cols
        self.NNT = (n + P - 1) // P
        self.LASTN = n - (self.NNT - 1) * P
        self.HIPAD = n + 1 - (LO_MAX + 1)     # hi-half index of pad row N+1


class EdgePlan:
    """Per-tile chunk shapes, shared across cores (max over cores)."""


def _mk_plan(cfg, deg, cost_ov=1.3):
    """deg: [2(half), NC, NT, P] int. Shared per-tile K0/OV (max over cores)."""
    NC, NT = cfg.NC, cfg.NT
    K0 = np.zeros((2, NC, NT), np.int64)
    for h in range(2):
        for c in range(NC):
            for t in range(NT):
                d = deg[h, c, t]
                best = None
                for k0 in range(0, int(d.max()) + 1):
                    ov = int(np.ceil(np.maximum(d - k0, 0).sum() / P))
                    cost = k0 + cost_ov * ov
                    if best is None or cost < best[0]:
                        best = (cost, k0)
                K0[h, c, t] = best[1]
    K0s = K0.max(axis=1)                                    # [2, NT]
    ovn = np.ceil(np.maximum(deg - K0s[:, None, :, None], 0)
                  .sum(axis=3) / P).astype(np.int64)        # [2, NC, NT]
    OVs = ovn.max(axis=1)                                   # [2, NT]
    plan = EdgePlan()
    plan.K0L, plan.K0H = K0s[0], K0s[1]
    plan.OVL, plan.OVH = OVs[0], OVs[1]
    plan.CL = plan.OVL + plan.K0L            # chunks in the lo gather call
    plan.CH = plan.K0H + plan.OVH            # chunks in the hi gather call
    plan.S = plan.CL + plan.CH
    plan.NOV = plan.OVL + plan.OVH
    plan.key = tuple(map(int, np.concatenate(
        [plan.K0L, plan.K0H, plan.OVL, plan.OVH])))
    return plan


def _pack16(vals):
    # vals: [n_chunks*P] int16 in slot order j=k*128+p; idx j lives at
    # [j%16, j//16], replicated across the 8 stripes of 16 partitions.
    a = vals.reshape(-1, 16).T.astype(np.int16)   # [16, n/16]
    return np.tile(a, (8, 1))


def _prep_host(cfg, x, edge_index, W1, a_src1, a_dst1, b1, W2, a_src2,
               a_dst2, b2):
    N, H, HID = cfg.N, cfg.H, cfg.HID
    NPC, NT, NC = cfg.NPC, cfg.NT, cfg.NC

    src = np.asarray(edge_index[0], dtype=np.int64)
    dst = np.asarray(edge_index[1], dtype=np.int64)
    core_of = dst // NPC
    tile_of = (dst % NPC) // P
    part_of = (dst % NPC) % P
    is_hi = (src + 1 > LO_MAX).astype(np.int64)   # lo idx = src+1 <= 32767

    deg = np.zeros((2, NC, NT, P), np.int64)
    np.add.at(deg, (is_hi, core_of, tile_of, part_of), 1)
    plan = _mk_plan(cfg, deg)
    K0L, K0H, OVL, OVH = plan.K0L, plan.K0H, plan.OVL, plan.OVH
    CL, CH, S, NOV = plan.CL, plan.CH, plan.S, plan.NOV
    SCL, SCH, SNOV = int(CL.sum()), int(CH.sum()), int(NOV.sum())

    per_core = []
    for c in range(NC):
        m = core_of == c
        s_c, hi_c = src[m], is_hi[m]
        t_c, p_c = tile_of[m], part_of[m]
        ilo = np.zeros((P, max(SCL, 1) * 8), np.int16)
        ihi = np.zeros((P, max(SCH, 1) * 8), np.int16)
        msk = np.zeros((P, max(SNOV, 1) * P), np.float16)
        mskT = np.zeros((P, max(SNOV, 1) * P), np.float16)
        olo = ohi = omk = 0
        for t in range(NT):
            mt = t_c == t
            s_t, p_t, hi_t = s_c[mt], p_c[mt], hi_c[mt]
            k0l, k0h = int(K0L[t]), int(K0H[t])
            ovl, ovh = int(OVL[t]), int(OVH[t])
            cl, ch = int(CL[t]), int(CH[t])
            # chunk order: [ov-lo | aligned-lo | aligned-hi | ov-hi]
            vlo = np.zeros(max(cl, 1) * P, np.int64)        # pad -> row 0
            vhi = np.full(max(ch, 1) * P, cfg.HIPAD, np.int64)
            ov_lo, ov_hi = [], []
            for p in range(P):
                es = s_t[(p_t == p) & (hi_t == 0)] + 1          # lo idx
                nk = min(len(es), k0l)
                for k in range(nk):
                    vlo[(ovl + k) * P + p] = es[k]
                ov_lo.extend((int(s), p) for s in es[k0l:])
                es = s_t[(p_t == p) & (hi_t == 1)] - LO_MAX     # hi idx
                nk = min(len(es), k0h)
                for k in range(nk):
                    vhi[k * P + p] = es[k]
                ov_hi.extend((int(s), p) for s in es[k0h:])
            assert len(ov_lo) <= ovl * P and len(ov_hi) <= ovh * P
            for j, (s, p) in enumerate(ov_lo):
                jj, js = j // P, j % P
                vlo[jj * P + js] = s
                msk[js, (omk + jj) * P + p] = 1.0
                mskT[p, (omk + jj) * P + js] = 1.0
            for j, (s, p) in enumerate(ov_hi):
                jj, js = j // P, j % P
                vhi[(k0h + jj) * P + js] = s
                msk[js, (omk + ovl + jj) * P + p] = 1.0
                mskT[p, (omk + ovl + jj) * P + js] = 1.0
            if cl:
                ilo[:, olo * 8:(olo + cl) * 8] = _pack16(vlo)
            if ch:
                ihi[:, ohi * 8:(ohi + ch) * 8] = _pack16(vhi)
            olo += cl
            ohi += ch
            omk += ovl + ovh
        adrows = (1 + c * NPC + np.arange(NT)[None, :] * P
                  + np.arange(P)[:, None]).astype(np.int32)
        np.clip(adrows, 1, N, out=adrows)
        per_core.append({
            "idxlo": np.ascontiguousarray(ilo),
            "idxhi": np.ascontiguousarray(ihi),
            "msk": np.ascontiguousarray(msk),
            "mskT": np.ascontiguousarray(mskT),
            "adrows": np.ascontiguousarray(adrows),
        })

    # fused weight tables (host-computed, f16)
    W1f = np.asarray(W1, np.float32)
    asrc_blk = np.zeros((cfg.C1, H), np.float32)
    adst_blk = np.zeros((cfg.C1, H), np.float32)
    for h in range(H):
        asrc_blk[h * HID:(h + 1) * HID, h] = np.asarray(a_src1, np.float32)[h]
        adst_blk[h * HID:(h + 1) * HID, h] = np.asarray(a_dst1, np.float32)[h]
    w1ext = np.concatenate([W1f, W1f @ asrc_blk, W1f @ adst_blk], axis=1)
    W2f = np.asarray(W2, np.float32)
    w2ext = np.concatenate(
        [W2f, W2f @ np.asarray(a_src2, np.float32).reshape(-1, 1),
         W2f @ np.asarray(a_dst2, np.float32).reshape(-1, 1)], axis=1)

    b1bc = np.zeros((P, cfg.G1C), np.float32)
    b1bc[:, :cfg.C1] = np.asarray(b1, np.float32)[None, :]
    b2bc = np.zeros((P, cfg.G2C), np.float32)
    b2bc[:, :cfg.OUT] = np.asarray(b2, np.float32)[None, :]

    pad1 = np.zeros((2, cfg.G1W), np.float16)
    pad1[:, cfg.C1:cfg.C1 + H] = NEG_AS
    pad2 = np.zeros((2, cfg.G2W), np.float16)
    pad2[:, cfg.OUT:cfg.OUT + 1] = NEG_AS

    common = {
        "xT16": np.ascontiguousarray(np.asarray(x, np.float32).T
                                     .astype(np.float16)),
        "w1ext": np.ascontiguousarray(w1ext.astype(np.float16)),
        "w2ext": np.ascontiguousarray(w2ext.astype(np.float16)),
        "b1bc": b1bc, "b2bc": b2bc,
        "pad1": pad1, "pad2": pad2,
        "identh": np.eye(P, dtype=np.float16),
    }
    return plan, common, per_core


def _build(cfg, plan):
    N, H, HID, C1 = cfg.N, cfg.H, cfg.HID, cfg.C1
    NT, NPC, NNT = cfg.NT, cfg.NPC, cfg.NNT
    C2 = cfg.OUT
    G1W, G2W, G1C, G2C = cfg.G1W, cfg.G2W, cfg.G1C, cfg.G2C
    K0L, K0H = plan.K0L, plan.K0H
    OVL, OVH = plan.OVL, plan.OVH
    CL, CH, S, NOV = plan.CL, plan.CH, plan.S, plan.NOV
    SCL, SCH, SNOV = int(CL.sum()), int(CH.sum()), int(NOV.sum())
    HI0 = LO_MAX + 1

    nc = bacc.Bacc("TRN2", target_bir_lowering=False, debug=False,
                   num_devices=cfg.NC)

    def din(name, shape, dt=F32):
        return nc.dram_tensor(name, shape, dt, kind="ExternalInput").ap()

    xT16 = din("xT16", [P, N], F16)
    w1ext = din("w1ext", [P, G1C], F16)
    w2ext = din("w2ext", [C1, G2C], F16)
    b1bc = din("b1bc", [P, G1C])
    b2bc = din("b2bc", [P, G2C])
    pad1 = din("pad1", [2, G1W], F16)
    pad2 = din("pad2", [2, G2W], F16)
    identh = din("identh", [P, P], F16)
    idxlo = din("idxlo", [P, max(SCL, 1) * 8], I16)
    idxhi = din("idxhi", [P, max(SCH, 1) * 8], I16)
    mskD = din("msk", [P, max(SNOV, 1) * P], F16)
    mskTD = din("mskT", [P, max(SNOV, 1) * P], F16)
    adrows = din("adrows", [P, NT], I32)

    out = nc.dram_tensor("out", [NT * P, C2], F32, kind="ExternalOutput").ap()

    G1 = nc.dram_tensor("G1", [N + 2, G1W], F16).ap()
    G2 = nc.dram_tensor("G2", [N + 2, G2W], F16).ap()
    G2c = nc.dram_tensor("G2c", [NPC, G2W], F16).ap()
    G2cf = nc.dram_tensor("G2cf", [N, G2W], F16, addr_space="Shared").ap()
    PRE = nc.dram_tensor("PRE", [NT * P, C2], F32).ap()

    with tile.TileContext(nc) as tc:
        with (
            tc.tile_pool(name="const", bufs=1) as const,
            tc.tile_pool(name="g2p", bufs=NT) as g2p,
        ):
            w1sb = const.tile([P, G1C], F16)
            nc.sync.dma_start(out=w1sb[:], in_=w1ext)
            w2sb = const.tile([C1, G2C], F16)
            nc.sync.dma_start(out=w2sb[:], in_=w2ext)
            b1sb = const.tile([P, G1C], F32)
            nc.sync.dma_start(out=b1sb[:], in_=b1bc)
            b2sb = const.tile([P, G2C], F32)
            nc.sync.dma_start(out=b2sb[:], in_=b2bc)
            idsb = const.tile([P, P], F16)
            nc.sync.dma_start(out=idsb[:], in_=identh)
            padsb = const.tile([2, G1W], F16)
            nc.sync.dma_start(out=padsb[:], in_=pad1)
            pad2sb = const.tile([2, G2W], F16)
            nc.sync.dma_start(out=pad2sb[:], in_=pad2)
            adr = const.tile([P, NT], I32)
            nc.sync.dma_start(out=adr[:], in_=adrows)
            nc.sync.dma_start(out=G1[0:1, :], in_=padsb[0:1, :])
            nc.sync.dma_start(out=G1[N + 1:N + 2, :], in_=padsb[1:2, :])
            nc.sync.dma_start(out=G2[0:1, :], in_=pad2sb[0:1, :])
            nc.sync.dma_start(out=G2[N + 1:N + 2, :], in_=pad2sb[1:2, :])

            # ---- phase A: G1 node rows (replicated on every core) ---------
            with (
                tc.tile_pool(name="pa", bufs=3) as pa,
                tc.tile_pool(name="pap", bufs=3, space="PSUM") as pap,
            ):
                for i in range(NNT):
                    nn = P if i < NNT - 1 else cfg.LASTN
                    xt = pa.tile([P, P], F16, tag="xt")
                    nc.sync.dma_start(out=xt[:, :nn],
                                      in_=xT16[:, i * P:i * P + nn])
                    ps = pap.tile([P, G1C], F32, space="PSUM", tag="ps")
                    nc.tensor.matmul(ps[:nn, :], lhsT=xt[:, :nn], rhs=w1sb[:],
                                     start=True, stop=True)
                    g1h = pa.tile([P, G1C], F16, tag="g1h")
                    nc.vector.tensor_tensor(out=g1h[:nn, :], in0=ps[:nn, :],
                                            in1=b1sb[:nn, :], op=OP.add)
                    nc.sync.dma_start(out=G1[1 + i * P:1 + i * P + nn, 0:G1C],
                                      in_=g1h[:nn, :])

            # ---- phase B: layer-1 edges + inline layer-2 node rows --------
            with (
                tc.tile_pool(name="pbig", bufs=2) as pbig,
                tc.tile_pool(name="pmed", bufs=2) as pmed,
                tc.tile_pool(name="pmsk", bufs=2) as pmsk,
                tc.tile_pool(name="pbp", bufs=2, space="PSUM") as pbp,
                tc.tile_pool(name="pbpa", bufs=2, space="PSUM") as pbpa,
                tc.tile_pool(name="pcp", bufs=2, space="PSUM") as pcp,
                tc.tile_pool(name="pcpt", bufs=2, space="PSUM") as pcpt,
            ):
                g2_tiles = []
                olo = ohi = omk = 0
                for t in range(NT):
                    ndst = P if t < NT - 1 else cfg.LAST
                    k0l, k0h = int(K0L[t]), int(K0H[t])
                    ovl, ovh = int(OVL[t]), int(OVH[t])
                    cl, ch, st = int(CL[t]), int(CH[t]), int(S[t])
                    nov = ovl + ovh
                    nal = k0l + k0h
                    ov_ids = list(range(ovl)) + list(range(st - ovh, st))

                    # self rows (own dsts): values + as/ad for the broadcast
                    vs = pmed.tile([P, G1C], F16, tag="vs")
                    nc.gpsimd.indirect_dma_start(
                        out=vs[:], out_offset=None, in_=G1,
                        in_offset=IndirectOffsetOnAxis(ap=adr[:, t:t + 1],
                                                       axis=0),
                        element_offset=0)

                    vg = pbig.tile([P, st * G1W], F16, tag="vg")
                    vg3 = vg[:].rearrange("p (k c) -> p k c", c=G1W)
                    if cl:
                        ilo = pmed.tile([P, cl * 8], I16, tag="ilo")
                        nc.sync.dma_start(
                            out=ilo[:],
                            in_=idxlo[:, olo * 8:(olo + cl) * 8])
                        nc.gpsimd.dma_gather(
                            out_ap=vg3[:, 0:cl, :], in_ap=G1[0:HI0, :],
                            idxs_ap=ilo[:], num_idxs=cl * P,
                            num_idxs_reg=cl * P, elem_size=G1W,
                            single_packet=False)
                    if ch:
                        ihi = pmed.tile([P, ch * 8], I16, tag="ihi")
                        nc.sync.dma_start(
                            out=ihi[:],
                            in_=idxhi[:, ohi * 8:(ohi + ch) * 8])
                        nc.gpsimd.dma_gather(
                            out_ap=vg3[:, cl:st, :], in_ap=G1[HI0:N + 2, :],
                            idxs_ap=ihi[:], num_idxs=ch * P,
                            num_idxs_reg=ch * P, elem_size=G1W,
                            single_packet=False)
                    if nov:
                        mk = pmsk.tile([P, nov * P], F16, tag="mk")
                        nc.sync.dma_start(
                            out=mk[:], in_=mskD[:, omk * P:(omk + nov) * P])
                        mkT = pmsk.tile([P, nov * P], F16, tag="mkT")
                        nc.sync.dma_start(
                            out=mkT[:], in_=mskTD[:, omk * P:(omk + nov) * P])

                    alp = pmed.tile([P, (st + 1) * H], F32, tag="alp")
                    alp3 = alp[:].rearrange("p (k h) -> p k h", h=H)
                    if nal:
                        nc.vector.tensor_tensor(
                            out=alp3[:, ovl:ovl + nal, :],
                            in0=vg3[:, ovl:ovl + nal, C1:C1 + H],
                            in1=vs[:, C1 + H:C1 + 2 * H]
                                .rearrange("p (o h) -> p o h", o=1)
                                .to_broadcast([P, nal, H]),
                            op=OP.add)
                    nc.vector.tensor_tensor(
                        out=alp3[:, st, :], in0=vs[:, C1:C1 + H],
                        in1=vs[:, C1 + H:C1 + 2 * H], op=OP.add)
                    for j, k in enumerate(ov_ids):
                        adp = pbpa.tile([P, H], F32, space="PSUM", tag="adp")
                        nc.tensor.matmul(adp[:],
                                         lhsT=mkT[:, j * P:(j + 1) * P],
                                         rhs=vs[:, C1 + H:C1 + 2 * H],
                                         start=True, stop=True)
                        nc.vector.tensor_tensor(
                            out=alp3[:, k, :], in0=vg3[:, k, C1:C1 + H],
                            in1=adp[:], op=OP.add)

                    # e = exp(lrelu(alpha)); pad slots give e = 0
                    asc = pmed.tile([P, (st + 1) * H], F32, tag="asc")
                    nc.vector.tensor_scalar(out=asc[:], in0=alp[:],
                                            scalar1=cfg.NEG, scalar2=None,
                                            op0=OP.mult)
                    lrl = pmed.tile([P, (st + 1) * H], F32, tag="lrl")
                    nc.vector.tensor_tensor(out=lrl[:], in0=alp[:], in1=asc[:],
                                            op=OP.max)
                    eeh = pmed.tile([P, (st + 1) * H], F16, tag="eeh")
                    nc.scalar.activation(out=eeh[:], in_=lrl[:], func=AF.Exp)
                    eeh3 = eeh[:].rearrange("p (k h) -> p k h", h=H)

                    # rhs = [e*h | e] for all chunks + self
                    xx = pbig.tile([P, (st + 1) * (C1 + H)], F16, tag="xx")
                    xx3 = xx[:].rearrange("p (k c) -> p k c", c=C1 + H)
                    nc.vector.tensor_copy(out=xx3[:, :, C1:C1 + H], in_=eeh3)
                    nc.vector.tensor_tensor(
                        out=xx3[:, 0:st, 0:C1]
                            .rearrange("p k (h c) -> p k h c", c=HID),
                        in0=vg3[:, :, 0:C1]
                            .rearrange("p k (h c) -> p k h c", c=HID),
                        in1=eeh[:, 0:st * H]
                            .rearrange("p (k h o) -> p k h o", h=H, o=1)
                            .to_broadcast([P, st, H, HID]),
                        op=OP.mult)
                    nc.vector.tensor_tensor(
                        out=xx3[:, st, 0:C1]
                            .rearrange("p (h c) -> p h c", c=HID),
                        in0=vs[:, 0:C1].rearrange("p (h c) -> p h c", c=HID),
                        in1=eeh3[:, st, :].rearrange("p (h o) -> p h o", o=1)
                            .to_broadcast([P, H, HID]),
                        op=OP.mult)

                    ps = pbp.tile([P, C1 + H], F32, space="PSUM", tag="ps")
                    for k in range(st + 1):
                        if k in ov_ids:
                            j = ov_ids.index(k)
                            lhsT = mk[:, j * P:(j + 1) * P]
                        else:
                            lhsT = idsb[:]
                        nc.tensor.matmul(ps[:], lhsT=lhsT, rhs=xx3[:, k, :],
                                         start=(k == 0), stop=(k == st))
                    rec = pmed.tile([P, H], F32, tag="rec")
                    nc.vector.reciprocal(out=rec[:ndst, :],
                                         in_=ps[:ndst, C1:C1 + H])
                    o1 = pmed.tile([P, C1], F16, tag="o1")
                    if ndst < P:
                        nc.vector.memset(o1[:], 0.0)
                    for h in range(H):
                        nc.vector.tensor_scalar(
                            out=o1[:ndst, h * HID:(h + 1) * HID],
                            in0=ps[:ndst, h * HID:(h + 1) * HID],
                            scalar1=rec[:ndst, h:h + 1], scalar2=0.0,
                            op0=OP.mult, op1=OP.max)

                    # inline layer-2 node row: h2 = o1 @ W2ext + b2
                    tp = pcpt.tile([P, P], F16, space="PSUM", tag="tp")
                    nc.tensor.transpose(out=tp[:], in_=o1[:], identity=idsb[:])
                    o1t = pmed.tile([P, P], F16, tag="o1t")
                    nc.vector.tensor_copy(out=o1t[:], in_=tp[:])
                    hp = pcp.tile([P, G2C], F32, space="PSUM", tag="hp")
                    nc.tensor.matmul(hp[:], lhsT=o1t[:], rhs=w2sb[:],
                                     start=True, stop=True)
                    g2f = g2p.tile([P, G2W], F16, tag="g2f")
                    g2_tiles.append(g2f)
                    nc.vector.tensor_tensor(out=g2f[:, 0:G2C], in0=hp[:],
                                            in1=b2sb[:], op=OP.add)
                    nc.sync.dma_start(out=G2c[t * P:t * P + ndst, :],
                                      in_=g2f[:ndst, :])

                    olo += cl
                    ohi += ch
                    omk += nov

            nc.gpsimd.collective_compute(
                "AllGather", OP.bypass,
                replica_groups=[list(range(cfg.NC))],
                ins=[G2c], outs=[G2cf])
            nc.sync.dma_start(out=G2[1:N + 1, :], in_=G2cf[:, :])

            # ---- phase D: layer-2 edge aggregation (1 head) ---------------
            with (
                tc.tile_pool(name="pdig", bufs=2) as pdig,
                tc.tile_pool(name="pdmd", bufs=2) as pdmd,
                tc.tile_pool(name="pdmk", bufs=2) as pdmk,
                tc.tile_pool(name="pdp", bufs=2, space="PSUM") as pdp,
                tc.tile_pool(name="pdpa", bufs=2, space="PSUM") as pdpa,
            ):
                olo = ohi = omk = 0
                for t in range(NT):
                    ndst = P if t < NT - 1 else cfg.LAST
                    k0l, k0h = int(K0L[t]), int(K0H[t])
                    ovl, ovh = int(OVL[t]), int(OVH[t])
                    cl, ch, st = int(CL[t]), int(CH[t]), int(S[t])
                    nov = ovl + ovh
                    nal = k0l + k0h
                    ov_ids = list(range(ovl)) + list(range(st - ovh, st))
                    g2f = g2_tiles[t]         # SBUF-resident own rows

                    vg = pdig.tile([P, st * G2W], F16, tag="vg2")
                    vg3 = vg[:].rearrange("p (k c) -> p k c", c=G2W)
                    if cl:
                        ilo = pdmd.tile([P, cl * 8], I16, tag="ilo2")
                        nc.sync.dma_start(
                            out=ilo[:],
                            in_=idxlo[:, olo * 8:(olo + cl) * 8])
                        nc.gpsimd.dma_gather(
                            out_ap=vg3[:, 0:cl, :], in_ap=G2[0:HI0, :],
                            idxs_ap=ilo[:], num_idxs=cl * P,
                            num_idxs_reg=cl * P, elem_size=G2W,
                            single_packet=False)
                    if ch:
                        ihi = pdmd.tile([P, ch * 8], I16, tag="ihi2")
                        nc.sync.dma_start(
                            out=ihi[:],
                            in_=idxhi[:, ohi * 8:(ohi + ch) * 8])
                        nc.gpsimd.dma_gather(
                            out_ap=vg3[:, cl:st, :], in_ap=G2[HI0:N + 2, :],
                            idxs_ap=ihi[:], num_idxs=ch * P,
                            num_idxs_reg=ch * P, elem_size=G2W,
                            single_packet=False)
                    if nov:
                        mk = pdmk.tile([P, nov * P], F16, tag="mk2")
                        nc.sync.dma_start(
                            out=mk[:], in_=mskD[:, omk * P:(omk + nov) * P])
                        mkT = pdmk.tile([P, nov * P], F16, tag="mkT2")
                        nc.sync.dma_start(
                            out=mkT[:], in_=mskTD[:, omk * P:(omk + nov) * P])

                    alp = pdmd.tile([P, st + 1], F32, tag="alp2")
                    alp3 = alp[:].rearrange("p (k o) -> p k o", o=1)
                    if nal:
                        nc.vector.tensor_tensor(
                            out=alp3[:, ovl:ovl + nal, :],
                            in0=vg3[:, ovl:ovl + nal, C2:C2 + 1],
                            in1=g2f[:, C2 + 1:C2 + 2]
                                .rearrange("p (o h) -> p o h", o=1)
                                .to_broadcast([P, nal, 1]),
                            op=OP.add)
                    nc.vector.tensor_tensor(
                        out=alp3[:, st, :], in0=g2f[:, C2:C2 + 1],
                        in1=g2f[:, C2 + 1:C2 + 2], op=OP.add)
                    for j, k in enumerate(ov_ids):
                        adp = pdpa.tile([P, 1], F32, space="PSUM", tag="adp2")
                        nc.tensor.matmul(adp[:],
                                         lhsT=mkT[:, j * P:(j + 1) * P],
                                         rhs=g2f[:, C2 + 1:C2 + 2],
                                         start=True, stop=True)
                        nc.vector.tensor_tensor(
                            out=alp3[:, k, :], in0=vg3[:, k, C2:C2 + 1],
                            in1=adp[:], op=OP.add)

                    asc = pdmd.tile([P, st + 1], F32, tag="asc2")
                    nc.vector.tensor_scalar(out=asc[:], in0=alp[:],
                                            scalar1=cfg.NEG, scalar2=None,
                                            op0=OP.mult)
                    lrl = pdmd.tile([P, st + 1], F32, tag="lrl2")
                    nc.vector.tensor_tensor(out=lrl[:], in0=alp[:], in1=asc[:],
                                            op=OP.max)
                    eeh = pdmd.tile([P, st + 1], F16, tag="eeh2")
                    nc.scalar.activation(out=eeh[:], in_=lrl[:], func=AF.Exp)

                    xx = pdig.tile([P, (st + 1) * (C2 + 1)], F16, tag="xx2")
                    xx3 = xx[:].rearrange("p (k c) -> p k c", c=C2 + 1)
                    nc.vector.tensor_copy(
                        out=xx3[:, :, C2:C2 + 1],
                        in_=eeh[:].rearrange("p (k o) -> p k o", o=1))
                    nc.vector.tensor_tensor(
                        out=xx3[:, 0:st, 0:C2],
                        in0=vg3[:, :, 0:C2],
                        in1=eeh[:, 0:st].rearrange("p (k o) -> p k o", o=1)
                            .to_broadcast([P, st, C2]),
                        op=OP.mult)
                    nc.vector.tensor_tensor(
                        out=xx3[:, st, 0:C2],
                        in0=g2f[:, 0:C2],
                        in1=eeh[:, st:st + 1].to_broadcast([P, C2]),
                        op=OP.mult)

                    ps = pdp.tile([P, C2 + 1], F32, space="PSUM", tag="ps2")
                    for k in range(st + 1):
                        if k in ov_ids:
                            j = ov_ids.index(k)
                            lhsT = mk[:, j * P:(j + 1) * P]
                        else:
                            lhsT = idsb[:]
                        nc.tensor.matmul(ps[:], lhsT=lhsT, rhs=xx3[:, k, :],
                                         start=(k == 0), stop=(k == st))
                    rec = pdmd.tile([P, 1], F32, tag="rec2")
                    nc.vector.reciprocal(out=rec[:ndst, :],
                                         in_=ps[:ndst, C2:C2 + 1])
                    o2 = pdmd.tile([P, C2], F32, tag="o2")
                    nc.vector.tensor_scalar(out=o2[:ndst, :],
                                            in0=ps[:ndst, 0:C2],
                                            scalar1=rec[:ndst, :],
                                            scalar2=None, op0=OP.mult)
                    nc.sync.dma_start(out=PRE[t * P:t * P + ndst, :],
                                      in_=o2[:ndst, :])

                    olo += cl
                    ohi += ch
                    omk += nov

            # ---- phase E: one sigmoid sweep -------------------------------
            with tc.tile_pool(name="pe", bufs=2) as pe:
                FW = NT * C2
                pre_f = PRE.rearrange("(a b) c -> a (b c)", a=P)
                out_f = out.rearrange("(a b) c -> a (b c)", a=P)
                pei = pe.tile([P, FW], F32, tag="pei")
                nc.sync.dma_start(out=pei[:], in_=pre_f)
                peo = pe.tile([P, FW], F32, tag="peo")
                nc.scalar.activation(out=peo[:], in_=pei[:], func=AF.Sigmoid)
                nc.sync.dma_start(out=out_f, in_=peo[:])

    nc.compile()
    return nc


_CACHE: dict = {}


def _get_module(cfg, plan):
    key = (cfg.N, cfg.E, plan.key)
    if key not in _CACHE:
        _CACHE[key] = _build(cfg, plan)
    return _CACHE[key]


def _run(cfg, inputs, trace=False):
    plan, common, per_core = _prep_host(
        cfg, inputs["x"], inputs["edge_index"], inputs["W1"],
        inputs["a_src1"], inputs["a_dst1"], inputs["b1"], inputs["W2"],
        inputs["a_src2"], inputs["a_dst2"], inputs["b2"])
    nc = _get_module(cfg, plan)
    in_maps = [dict(common, **pc) for pc in per_core]
    res = run_bass_kernel_spmd(nc, in_maps, core_ids=list(range(cfg.NC)),
                               trace=trace)
    shards = [np.asarray(res.results[c]["out"])[:cfg.NPC]
              for c in range(cfg.NC)]
    full = np.concatenate(shards, axis=0).astype(np.float32)
    return (full, res) if trace else full


def kernel(**inputs) -> np.ndarray:
    cfg = GATConfig(n=50000, e=800000)
    return _run(cfg, inputs)


# revision 13
# speedup vs baseline: 1.4832x; 1.2647x over previous
"""GAT 2-layer propagation kernel for Trainium2, 8 NeuronCores (SPMD).

Strategy (edge-parallel, dst-node-range sharded across 8 cores):
  - Core c owns dst nodes [c*6250, (c+1)*6250); edges go to the core owning
    their dst, so each core emits its contiguous output rows; one AllGather
    per layer replicates the next layer's node table.
  - Per layer a DRAM table holds one row per node plus two PAD rows:
      G1 = [padlo | rows [h x128 | as x4 | ad x4 | junk] | padhi]  512 B/row
      G2 = [padlo | rows [h2 x32 | as2 | ad2 | junk]     | padhi]  256 B/row
    Pad rows carry as = -30000 so padded gather slots produce e = 0 after
    exp(leakyrelu(.)) — no separate validity mask or multiply. h carries the
    layer bias folded in (softmax weights sum to 1).
  - Per 128-dst tile, edges live in "chunks" of 128 slots (partition dim)
    with PER-TILE chunk counts (shapes shared across cores = max over the 8
    cores, so one BIR serves all SPMD cores; slot padding points at the pad
    rows). Aligned chunks put dst p's k-th edge at partition p: the
    segment-sum matmul uses an identity stationary operand and alpha_dst
    broadcasts from the tile's self-row columns. Overflow edges (degree >
    K0) go to densely packed chunks whose one-hot masks (and transposes,
    for the alpha_dst matmul) are built on the HOST and DMA'd in.
  - Self-loop edges are NOT gathered: layer 1 fetches each tile's own rows
    with one small indirect DMA (per-core row offsets are runtime inputs),
    which also provides as/ad columns for the aligned broadcast; layer 2
    keeps its own rows SBUF-resident from the inline layer-2 row phase.
  - Value rows are fetched with dma_gather (int16 indices) in two calls per
    tile over the table halves [0, 32768) and [32768, N+2); the SAME index
    and mask tables drive both layers. dma_gather descriptor generation on
    GpSimd (~8 ns/slot) is the critical resource; everything else overlaps
    under it.
  - e = exp(leakyrelu(as+ad)) needs no max-subtraction (|alpha| is far from
    fp32 exp overflow). Aggregation accumulates [sum e*h | sum e] in fp32
    PSUM; the softmax division happens once per dst. Layer-2 node rows are
    computed inline per tile (padded to 256 B so the AllGather output is
    gatherable directly after one dense copy), and the final sigmoid runs
    as one deferred sweep.
"""

import numpy as np

import concourse.bacc as bacc
import concourse.tile as tile
from concourse import mybir
from concourse.bass import IndirectOffsetOnAxis
from concourse.bass_utils import run_bass_kernel_spmd

F32 = mybir.dt.float32
F16 = mybir.dt.float16
I32 = mybir.dt.int32
I16 = mybir.dt.int16
AF = mybir.ActivationFunctionType
OP = mybir.AluOpType

P = 128
LO_MAX = 32767          # lo half rows 0..32767; node n lo iff n+1 <= 32767
NEG_AS = -30000.0       # pad-row alpha_src; exp(lrelu(NEG_AS+ad)) == 0 in f32


class GATConfig:
    def __init__(self, n, e, in_dim=128, hid=32, heads=4, out_dim=32,
                 neg_slope=0.2, n_cores=8):
        assert in_dim == P and heads * hid == P
        self.N, self.E = n, e
        self.HID = hid
        self.H = heads
        self.OUT = out_dim
        self.NEG = neg_slope
        self.NC = n_cores
        assert n % n_cores == 0
        self.NPC = n // n_cores
        self.NT = (self.NPC + P - 1) // P
        self.LAST = self.NPC - (self.NT - 1) * P
        self.C1 = heads * hid                 # 128
        self.G1W = 256                        # f16 els/row (512 B)
        self.G2W = 128                        # f16 els/row (256 B)
        self.G1C = self.C1 + 2 * heads        # 136 used cols
        self.G2C = self.OUT + 2               # 34 used cols
        self.NNT = (n + P - 1) // P
        self.LASTN = n - (self.NNT - 1) * P
        self.HIPAD = n + 1 - (LO_MAX + 1)     # hi-half index of pad row N+1


class EdgePlan:
    """Per-tile chunk shapes, shared across cores (max over cores)."""


def _mk_plan(cfg, deg, cost_ov=1.0):
    """deg: [2(half), NC, NT, P] int. Shared per-tile K0/OV (max over cores)."""
    NC, NT = cfg.NC, cfg.NT
    K0 = np.zeros((2, NC, NT), np.int64)
    for h in range(2):
        for c in range(NC):
            for t in range(NT):
                d = deg[h, c, t]
                best = None
                for k0 in range(0, int(d.max()) + 1):
                    ov = int(np.ceil(np.maximum(d - k0, 0).sum() / P))
                    cost = k0 + cost_ov * ov
                    if best is None or cost < best[0]:
                        best = (cost, k0)
                K0[h, c, t] = best[1]
    K0s = K0.max(axis=1)                                    # [2, NT]
    ovn = np.ceil(np.maximum(deg - K0s[:, None, :, None], 0)
                  .sum(axis=3) / P).astype(np.int64)        # [2, NC, NT]
    OVs = ovn.max(axis=1)                                   # [2, NT]
    plan = EdgePlan()
    plan.K0L, plan.K0H = K0s[0], K0s[1]
    plan.OVL, plan.OVH = OVs[0], OVs[1]
    plan.CL = plan.OVL + plan.K0L            # chunks in the lo gather call
    plan.CH = plan.K0H + plan.OVH            # chunks in the hi gather call
    plan.S = plan.CL + plan.CH
    plan.NOV = plan.OVL + plan.OVH
    plan.key = tuple(map(int, np.concatenate(
        [plan.K0L, plan.K0H, plan.OVL, plan.OVH])))
    return plan


def _pack16(vals):
    # vals: [n_chunks*P] int16 in slot order j=k*128+p; idx j lives at
    # [j%16, j//16], replicated across the 8 stripes of 16 partitions.
    a = vals.reshape(-1, 16).T.astype(np.int16)   # [16, n/16]
    return np.tile(a, (8, 1))


def _prep_host(cfg, x, edge_index, W1, a_src1, a_dst1, b1, W2, a_src2,
               a_dst2, b2):
    N, H, HID = cfg.N, cfg.H, cfg.HID
    NPC, NT, NC = cfg.NPC, cfg.NT, cfg.NC

    src = np.asarray(edge_index[0], dtype=np.int64)
    dst = np.asarray(edge_index[1], dtype=np.int64)
    core_of = dst // NPC
    tile_of = (dst % NPC) // P
    part_of = (dst % NPC) % P
    is_hi = (src + 1 > LO_MAX).astype(np.int64)   # lo idx = src+1 <= 32767

    deg = np.zeros((2, NC, NT, P), np.int64)
    np.add.at(deg, (is_hi, core_of, tile_of, part_of), 1)
    plan = _mk_plan(cfg, deg)
    K0L, K0H, OVL, OVH = plan.K0L, plan.K0H, plan.OVL, plan.OVH
    CL, CH, S, NOV = plan.CL, plan.CH, plan.S, plan.NOV
    SCL, SCH, SNOV = int(CL.sum()), int(CH.sum()), int(NOV.sum())

    per_core = []
    for c in range(NC):
        m = core_of == c
        s_c, hi_c = src[m], is_hi[m]
        t_c, p_c = tile_of[m], part_of[m]
        ilo = np.zeros((P, max(SCL, 1) * 8), np.int16)
        ihi = np.zeros((P, max(SCH, 1) * 8), np.int16)
        msk = np.zeros((P, max(SNOV, 1) * P), np.float16)
        mskT = np.zeros((P, max(SNOV, 1) * P), np.float16)
        olo = ohi = omk = 0
        for t in range(NT):
            mt = t_c == t
            s_t, p_t, hi_t = s_c[mt], p_c[mt], hi_c[mt]
            k0l, k0h = int(K0L[t]), int(K0H[t])
            ovl, ovh = int(OVL[t]), int(OVH[t])
            cl, ch = int(CL[t]), int(CH[t])
            # chunk order: [ov-lo | aligned-lo | aligned-hi | ov-hi]
            vlo = np.zeros(max(cl, 1) * P, np.int64)        # pad -> row 0
            vhi = np.full(max(ch, 1) * P, cfg.HIPAD, np.int64)
            ov_lo, ov_hi = [], []
            for p in range(P):
                es = s_t[(p_t == p) & (hi_t == 0)] + 1          # lo idx
                nk = min(len(es), k0l)
                for k in range(nk):
                    vlo[(ovl + k) * P + p] = es[k]
                ov_lo.extend((int(s), p) for s in es[k0l:])
                es = s_t[(p_t == p) & (hi_t == 1)] - LO_MAX     # hi idx
                nk = min(len(es), k0h)
                for k in range(nk):
                    vhi[k * P + p] = es[k]
                ov_hi.extend((int(s), p) for s in es[k0h:])
            assert len(ov_lo) <= ovl * P and len(ov_hi) <= ovh * P
            for j, (s, p) in enumerate(ov_lo):
                jj, js = j // P, j % P
                vlo[jj * P + js] = s
                msk[js, (omk + jj) * P + p] = 1.0
                mskT[p, (omk + jj) * P + js] = 1.0
            for j, (s, p) in enumerate(ov_hi):
                jj, js = j // P, j % P
                vhi[(k0h + jj) * P + js] = s
                msk[js, (omk + ovl + jj) * P + p] = 1.0
                mskT[p, (omk + ovl + jj) * P + js] = 1.0
            if cl:
                ilo[:, olo * 8:(olo + cl) * 8] = _pack16(vlo)
            if ch:
                ihi[:, ohi * 8:(ohi + ch) * 8] = _pack16(vhi)
            olo += cl
            ohi += ch
            omk += ovl + ovh
        adrows = (1 + c * NPC + np.arange(NT)[None, :] * P
                  + np.arange(P)[:, None]).astype(np.int32)
        np.clip(adrows, 1, N, out=adrows)
        per_core.append({
            "idxlo": np.ascontiguousarray(ilo),
            "idxhi": np.ascontiguousarray(ihi),
            "msk": np.ascontiguousarray(msk),
            "mskT": np.ascontiguousarray(mskT),
            "adrows": np.ascontiguousarray(adrows),
        })

    # fused weight tables (host-computed, f16)
    W1f = np.asarray(W1, np.float32)
    asrc_blk = np.zeros((cfg.C1, H), np.float32)
    adst_blk = np.zeros((cfg.C1, H), np.float32)
    for h in range(H):
        asrc_blk[h * HID:(h + 1) * HID, h] = np.asarray(a_src1, np.float32)[h]
        adst_blk[h * HID:(h + 1) * HID, h] = np.asarray(a_dst1, np.float32)[h]
    w1ext = np.concatenate([W1f, W1f @ asrc_blk, W1f @ adst_blk], axis=1)
    W2f = np.asarray(W2, np.float32)
    w2ext = np.concatenate(
        [W2f, W2f @ np.asarray(a_src2, np.float32).reshape(-1, 1),
         W2f @ np.asarray(a_dst2, np.float32).reshape(-1, 1)], axis=1)

    b1bc = np.zeros((P, cfg.G1C), np.float32)
    b1bc[:, :cfg.C1] = np.asarray(b1, np.float32)[None, :]
    b2bc = np.zeros((P, cfg.G2C), np.float32)
    b2bc[:, :cfg.OUT] = np.asarray(b2, np.float32)[None, :]

    pad1 = np.zeros((2, cfg.G1W), np.float16)
    pad1[:, cfg.C1:cfg.C1 + H] = NEG_AS
    pad2 = np.zeros((2, cfg.G2W), np.float16)
    pad2[:, cfg.OUT:cfg.OUT + 1] = NEG_AS

    common = {
        "xT16": np.ascontiguousarray(np.asarray(x, np.float32).T
                                     .astype(np.float16)),
        "w1ext": np.ascontiguousarray(w1ext.astype(np.float16)),
        "w2ext": np.ascontiguousarray(w2ext.astype(np.float16)),
        "b1bc": b1bc, "b2bc": b2bc,
        "pad1": pad1, "pad2": pad2,
        "identh": np.eye(P, dtype=np.float16),
    }
    return plan, common, per_core


def _build(cfg, plan):
    N, H, HID, C1 = cfg.N, cfg.H, cfg.HID, cfg.C1
    NT, NPC, NNT = cfg.NT, cfg.NPC, cfg.NNT
    C2 = cfg.OUT
    G1W, G2W, G1C, G2C = cfg.G1W, cfg.G2W, cfg.G1C, cfg.G2C
    K0L, K0H = plan.K0L, plan.K0H
    OVL, OVH = plan.OVL, plan.OVH
    CL, CH, S, NOV = plan.CL, plan.CH, plan.S, plan.NOV
    SCL, SCH, SNOV = int(CL.sum()), int(CH.sum()), int(NOV.sum())
    HI0 = LO_MAX + 1

    nc = bacc.Bacc("TRN2", target_bir_lowering=False, debug=False,
                   num_devices=cfg.NC)

    def din(name, shape, dt=F32):
        return nc.dram_tensor(name, shape, dt, kind="ExternalInput").ap()

    xT16 = din("xT16", [P, N], F16)
    w1ext = din("w1ext", [P, G1C], F16)
    w2ext = din("w2ext", [C1, G2C], F16)
    b1bc = din("b1bc", [P, G1C])
    b2bc = din("b2bc", [P, G2C])
    pad1 = din("pad1", [2, G1W], F16)
    pad2 = din("pad2", [2, G2W], F16)
    identh = din("identh", [P, P], F16)
    idxlo = din("idxlo", [P, max(SCL, 1) * 8], I16)
    idxhi = din("idxhi", [P, max(SCH, 1) * 8], I16)
    mskD = din("msk", [P, max(SNOV, 1) * P], F16)
    mskTD = din("mskT", [P, max(SNOV, 1) * P], F16)
    adrows = din("adrows", [P, NT], I32)

    out = nc.dram_tensor("out", [NT * P, C2], F32, kind="ExternalOutput").ap()

    G1 = nc.dram_tensor("G1", [N + 2, G1W], F16).ap()
    G2 = nc.dram_tensor("G2", [N + 2, G2W], F16).ap()
    # layer-2 rows are AllGathered in two halves so the first collective
    # overlaps with the second half of phase B
    SPLIT_T = NT // 2
    NA = SPLIT_T * P
    NB = NPC - NA
    G2cA = nc.dram_tensor("G2cA", [NA, G2W], F16).ap()
    G2cB = nc.dram_tensor("G2cB", [NB, G2W], F16).ap()
    G2cfA = nc.dram_tensor("G2cfA", [cfg.NC * NA, G2W], F16,
                           addr_space="Shared").ap()
    G2cfB = nc.dram_tensor("G2cfB", [cfg.NC * NB, G2W], F16,
                           addr_space="Shared").ap()
    PRE = nc.dram_tensor("PRE", [NT * P, C2], F32).ap()

    with tile.TileContext(nc) as tc:
        with (
            tc.tile_pool(name="const", bufs=1) as const,
            tc.tile_pool(name="g2p", bufs=NT) as g2p,
        ):
            w1sb = const.tile([P, G1C], F16)
            nc.sync.dma_start(out=w1sb[:], in_=w1ext)
            w2sb = const.tile([C1, G2C], F16)
            nc.sync.dma_start(out=w2sb[:], in_=w2ext)
            b1sb = const.tile([P, G1C], F32)
            nc.sync.dma_start(out=b1sb[:], in_=b1bc)
            b2sb = const.tile([P, G2C], F32)
            nc.sync.dma_start(out=b2sb[:], in_=b2bc)
            idsb = const.tile([P, P], F16)
            nc.sync.dma_start(out=idsb[:], in_=identh)
            padsb = const.tile([2, G1W], F16)
            nc.sync.dma_start(out=padsb[:], in_=pad1)
            pad2sb = const.tile([2, G2W], F16)
            nc.sync.dma_start(out=pad2sb[:], in_=pad2)
            adr = const.tile([P, NT], I32)
            nc.sync.dma_start(out=adr[:], in_=adrows)
            nc.sync.dma_start(out=G1[0:1, :], in_=padsb[0:1, :])
            nc.sync.dma_start(out=G1[N + 1:N + 2, :], in_=padsb[1:2, :])
            nc.sync.dma_start(out=G2[0:1, :], in_=pad2sb[0:1, :])
            nc.sync.dma_start(out=G2[N + 1:N + 2, :], in_=pad2sb[1:2, :])

            # ---- phase A: G1 node rows (replicated on every core) ---------
            # grouped 8 tiles per DMA so transfers are ~256 KB, not 32 KB
            GA = 8
            with (
                tc.tile_pool(name="pa", bufs=3) as pa,
                tc.tile_pool(name="pap", bufs=3, space="PSUM") as pap,
            ):
                for g in range((NNT + GA - 1) // GA):
                    i0 = g * GA
                    gw = min(GA, NNT - i0)
                    nodes = min(N, (i0 + gw) * P) - i0 * P
                    xt = pa.tile([P, GA * P], F16, tag="xt")
                    nc.sync.dma_start(out=xt[:, :nodes],
                                      in_=xT16[:, i0 * P:i0 * P + nodes])
                    g1h = pa.tile([P, GA * G1C], F16, tag="g1h")
                    for j in range(gw):
                        nn = min(P, nodes - j * P)
                        ps = pap.tile([P, G1C], F32, space="PSUM", tag="ps")
                        nc.tensor.matmul(ps[:nn, :], lhsT=xt[:, j * P:j * P + nn],
                                         rhs=w1sb[:], start=True, stop=True)
                        nc.vector.tensor_tensor(
                            out=g1h[:nn, j * G1C:(j + 1) * G1C],
                            in0=ps[:nn, :], in1=b1sb[:nn, :], op=OP.add)
                    if gw == GA and nodes == GA * P:
                        nc.sync.dma_start(
                            out=G1[1 + i0 * P:1 + (i0 + GA) * P, 0:G1C]
                                .rearrange("(g p) c -> p g c", g=GA),
                            in_=g1h[:].rearrange("p (g c) -> p g c", g=GA))
                    else:
                        for j in range(gw):
                            nn = min(P, nodes - j * P)
                            nc.sync.dma_start(
                                out=G1[1 + (i0 + j) * P:
                                       1 + (i0 + j) * P + nn, 0:G1C],
                                in_=g1h[:nn, j * G1C:(j + 1) * G1C])

            # ---- phase B: layer-1 edges + inline layer-2 node rows --------
            with (
                tc.tile_pool(name="pbig", bufs=2) as pbig,
                tc.tile_pool(name="pmed", bufs=2) as pmed,
                tc.tile_pool(name="pmsk", bufs=2) as pmsk,
                tc.tile_pool(name="pbp", bufs=2, space="PSUM") as pbp,
                tc.tile_pool(name="pbpa", bufs=2, space="PSUM") as pbpa,
                tc.tile_pool(name="pcp", bufs=2, space="PSUM") as pcp,
                tc.tile_pool(name="pcpt", bufs=2, space="PSUM") as pcpt,
            ):
                g2_tiles = []
                olo = ohi = omk = 0
                for t in range(NT):
                    ndst = P if t < NT - 1 else cfg.LAST
                    k0l, k0h = int(K0L[t]), int(K0H[t])
                    ovl, ovh = int(OVL[t]), int(OVH[t])
                    cl, ch, st = int(CL[t]), int(CH[t]), int(S[t])
                    nov = ovl + ovh
                    nal = k0l + k0h
                    ov_ids = list(range(ovl)) + list(range(st - ovh, st))

                    # self rows (own dsts): values + as/ad for the broadcast
                    vs = pmed.tile([P, G1C], F16, tag="vs")
                    nc.gpsimd.indirect_dma_start(
                        out=vs[:], out_offset=None, in_=G1,
                        in_offset=IndirectOffsetOnAxis(ap=adr[:, t:t + 1],
                                                       axis=0),
                        element_offset=0)

                    vg = pbig.tile([P, st * G1W], F16, tag="vg")
                    vg3 = vg[:].rearrange("p (k c) -> p k c", c=G1W)
                    if cl:
                        ilo = pmed.tile([P, cl * 8], I16, tag="ilo")
                        nc.sync.dma_start(
                            out=ilo[:],
                            in_=idxlo[:, olo * 8:(olo + cl) * 8])
                        nc.gpsimd.dma_gather(
                            out_ap=vg3[:, 0:cl, :], in_ap=G1[0:HI0, :],
                            idxs_ap=ilo[:], num_idxs=cl * P,
                            num_idxs_reg=cl * P, elem_size=G1W,
                            single_packet=False)
                    if ch:
                        ihi = pmed.tile([P, ch * 8], I16, tag="ihi")
                        nc.sync.dma_start(
                            out=ihi[:],
                            in_=idxhi[:, ohi * 8:(ohi + ch) * 8])
                        nc.gpsimd.dma_gather(
                            out_ap=vg3[:, cl:st, :], in_ap=G1[HI0:N + 2, :],
                            idxs_ap=ihi[:], num_idxs=ch * P,
                            num_idxs_reg=ch * P, elem_size=G1W,
                            single_packet=False)
                    if nov:
                        mk = pmsk.tile([P, nov * P], F16, tag="mk")
                        nc.sync.dma_start(
                            out=mk[:], in_=mskD[:, omk * P:(omk + nov) * P])
                        mkT = pmsk.tile([P, nov * P], F16, tag="mkT")
                        nc.sync.dma_start(
                            out=mkT[:], in_=mskTD[:, omk * P:(omk + nov) * P])

                    alp = pmed.tile([P, (st + 1) * H], F32, tag="alp")
                    alp3 = alp[:].rearrange("p (k h) -> p k h", h=H)
                    if nal:
                        nc.vector.tensor_tensor(
                            out=alp3[:, ovl:ovl + nal, :],
                            in0=vg3[:, ovl:ovl + nal, C1:C1 + H],
                            in1=vs[:, C1 + H:C1 + 2 * H]
                                .rearrange("p (o h) -> p o h", o=1)
                                .to_broadcast([P, nal, H]),
                            op=OP.add)
                    nc.vector.tensor_tensor(
                        out=alp3[:, st, :], in0=vs[:, C1:C1 + H],
                        in1=vs[:, C1 + H:C1 + 2 * H], op=OP.add)
                    for j, k in enumerate(ov_ids):
                        adp = pbpa.tile([P, H], F32, space="PSUM", tag="adp")
                        nc.tensor.matmul(adp[:],
                                         lhsT=mkT[:, j * P:(j + 1) * P],
                                         rhs=vs[:, C1 + H:C1 + 2 * H],
                                         start=True, stop=True)
                        nc.vector.tensor_tensor(
                            out=alp3[:, k, :], in0=vg3[:, k, C1:C1 + H],
                            in1=adp[:], op=OP.add)

                    # e = exp(lrelu(alpha)); pad slots give e = 0
                    asc = pmed.tile([P, (st + 1) * H], F32, tag="asc")
                    nc.vector.tensor_scalar(out=asc[:], in0=alp[:],
                                            scalar1=cfg.NEG, scalar2=None,
                                            op0=OP.mult)
                    lrl = pmed.tile([P, (st + 1) * H], F32, tag="lrl")
                    nc.vector.tensor_tensor(out=lrl[:], in0=alp[:], in1=asc[:],
                                            op=OP.max)
                    eeh = pmed.tile([P, (st + 1) * H], F16, tag="eeh")
                    nc.scalar.activation(out=eeh[:], in_=lrl[:], func=AF.Exp)
                    eeh3 = eeh[:].rearrange("p (k h) -> p k h", h=H)

                    # rhs = [e*h | e] for all chunks + self
                    xx = pbig.tile([P, (st + 1) * (C1 + H)], F16, tag="xx")
                    xx3 = xx[:].rearrange("p (k c) -> p k c", c=C1 + H)
                    nc.vector.tensor_copy(out=xx3[:, :, C1:C1 + H], in_=eeh3)
                    nc.vector.tensor_tensor(
                        out=xx3[:, 0:st, 0:C1]
                            .rearrange("p k (h c) -> p k h c", c=HID),
                        in0=vg3[:, :, 0:C1]
                            .rearrange("p k (h c) -> p k h c", c=HID),
                        in1=eeh[:, 0:st * H]
                            .rearrange("p (k h o) -> p k h o", h=H, o=1)
                            .to_broadcast([P, st, H, HID]),
                        op=OP.mult)
                    nc.vector.tensor_tensor(
                        out=xx3[:, st, 0:C1]
                            .rearrange("p (h c) -> p h c", c=HID),
                        in0=vs[:, 0:C1].rearrange("p (h c) -> p h c", c=HID),
                        in1=eeh3[:, st, :].rearrange("p (h o) -> p h o", o=1)
                            .to_broadcast([P, H, HID]),
                        op=OP.mult)

                    ps = pbp.tile([P, C1 + H], F32, space="PSUM", tag="ps")
                    for k in range(st + 1):
                        if k in ov_ids:
                            j = ov_ids.index(k)
                            lhsT = mk[:, j * P:(j + 1) * P]
                        else:
                            lhsT = idsb[:]
                        nc.tensor.matmul(ps[:], lhsT=lhsT, rhs=xx3[:, k, :],
                                         start=(k == 0), stop=(k == st))
                    rec = pmed.tile([P, H], F32, tag="rec")
                    nc.vector.reciprocal(out=rec[:ndst, :],
                                         in_=ps[:ndst, C1:C1 + H])
                    o1 = pmed.tile([P, C1], F16, tag="o1")
                    if ndst < P:
                        nc.vector.memset(o1[:], 0.0)
                    for h in range(H):
                        nc.vector.tensor_scalar(
                            out=o1[:ndst, h * HID:(h + 1) * HID],
                            in0=ps[:ndst, h * HID:(h + 1) * HID],
                            scalar1=rec[:ndst, h:h + 1], scalar2=0.0,
                            op0=OP.mult, op1=OP.max)

                    # inline layer-2 node row: h2 = o1 @ W2ext + b2
                    tp = pcpt.tile([P, P], F16, space="PSUM", tag="tp")
                    nc.tensor.transpose(out=tp[:], in_=o1[:], identity=idsb[:])
                    o1t = pmed.tile([P, P], F16, tag="o1t")
                    nc.vector.tensor_copy(out=o1t[:], in_=tp[:])
                    hp = pcp.tile([P, G2C], F32, space="PSUM", tag="hp")
                    nc.tensor.matmul(hp[:], lhsT=o1t[:], rhs=w2sb[:],
                                     start=True, stop=True)
                    g2f = g2p.tile([P, G2W], F16, tag="g2f")
                    g2_tiles.append(g2f)
                    nc.vector.tensor_tensor(out=g2f[:, 0:G2C], in0=hp[:],
                                            in1=b2sb[:], op=OP.add)
                    if t < SPLIT_T:
                        nc.sync.dma_start(out=G2cA[t * P:t * P + ndst, :],
                                          in_=g2f[:ndst, :])
                    else:
                        r0 = (t - SPLIT_T) * P
                        nc.sync.dma_start(out=G2cB[r0:r0 + ndst, :],
                                          in_=g2f[:ndst, :])
                    if t == SPLIT_T - 1:
                        nc.gpsimd.collective_compute(
                            "AllGather", OP.bypass,
                            replica_groups=[list(range(cfg.NC))],
                            ins=[G2cA], outs=[G2cfA])
                        for c8 in range(cfg.NC):
                            nc.sync.dma_start(
                                out=G2[1 + c8 * NPC:1 + c8 * NPC + NA, :],
                                in_=G2cfA[c8 * NA:(c8 + 1) * NA, :])

                    olo += cl
                    ohi += ch
                    omk += nov

            nc.gpsimd.collective_compute(
                "AllGather", OP.bypass,
                replica_groups=[list(range(cfg.NC))],
                ins=[G2cB], outs=[G2cfB])
            for c8 in range(cfg.NC):
                nc.sync.dma_start(
                    out=G2[1 + c8 * NPC + NA:1 + (c8 + 1) * NPC, :],
                    in_=G2cfB[c8 * NB:(c8 + 1) * NB, :])

            # ---- phase D: layer-2 edge aggregation (1 head) ---------------
            with (
                tc.tile_pool(name="pdig", bufs=2) as pdig,
                tc.tile_pool(name="pdmd", bufs=2) as pdmd,
                tc.tile_pool(name="pdmk", bufs=2) as pdmk,
                tc.tile_pool(name="pdp", bufs=2, space="PSUM") as pdp,
                tc.tile_pool(name="pdpa", bufs=2, space="PSUM") as pdpa,
            ):
                olo = ohi = omk = 0
                for t in range(NT):
                    ndst = P if t < NT - 1 else cfg.LAST
                    k0l, k0h = int(K0L[t]), int(K0H[t])
                    ovl, ovh = int(OVL[t]), int(OVH[t])
                    cl, ch, st = int(CL[t]), int(CH[t]), int(S[t])
                    nov = ovl + ovh
                    nal = k0l + k0h
                    ov_ids = list(range(ovl)) + list(range(st - ovh, st))
                    g2f = g2_tiles[t]         # SBUF-resident own rows

                    vg = pdig.tile([P, st * G2W], F16, tag="vg2")
                    vg3 = vg[:].rearrange("p (k c) -> p k c", c=G2W)
                    if cl:
                        ilo = pdmd.tile([P, cl * 8], I16, tag="ilo2")
                        nc.sync.dma_start(
                            out=ilo[:],
                            in_=idxlo[:, olo * 8:(olo + cl) * 8])
                        nc.gpsimd.dma_gather(
                            out_ap=vg3[:, 0:cl, :], in_ap=G2[0:HI0, :],
                            idxs_ap=ilo[:], num_idxs=cl * P,
                            num_idxs_reg=cl * P, elem_size=G2W,
                            single_packet=False)
                    if ch:
                        ihi = pdmd.tile([P, ch * 8], I16, tag="ihi2")
                        nc.sync.dma_start(
                            out=ihi[:],
                            in_=idxhi[:, ohi * 8:(ohi + ch) * 8])
                        nc.gpsimd.dma_gather(
                            out_ap=vg3[:, cl:st, :], in_ap=G2[HI0:N + 2, :],
                            idxs_ap=ihi[:], num_idxs=ch * P,
                            num_idxs_reg=ch * P, elem_size=G2W,
                            single_packet=False)
                    if nov:
                        mk = pdmk.tile([P, nov * P], F16, tag="mk2")
                        nc.sync.dma_start(
                            out=mk[:], in_=mskD[:, omk * P:(omk + nov) * P])
                        mkT = pdmk.tile([P, nov * P], F16, tag="mkT2")
                        nc.sync.dma_start(
                            out=mkT[:], in_=mskTD[:, omk * P:(omk + nov) * P])

                    alp = pdmd.tile([P, st + 1], F32, tag="alp2")
                    alp3 = alp[:].rearrange("p (k o) -> p k o", o=1)
                    if nal:
                        nc.vector.tensor_tensor(
                            out=alp3[:, ovl:ovl + nal, :],
                            in0=vg3[:, ovl:ovl + nal, C2:C2 + 1],
                            in1=g2f[:, C2 + 1:C2 + 2]
                                .rearrange("p (o h) -> p o h", o=1)
                                .to_broadcast([P, nal, 1]),
                            op=OP.add)
                    nc.vector.tensor_tensor(
                        out=alp3[:, st, :], in0=g2f[:, C2:C2 + 1],
                        in1=g2f[:, C2 + 1:C2 + 2], op=OP.add)
                    for j, k in enumerate(ov_ids):
                        adp = pdpa.tile([P, 1], F32, space="PSUM", tag="adp2")
                        nc.tensor.matmul(adp[:],
                                         lhsT=mkT[:, j * P:(j + 1) * P],
                                         rhs=g2f[:, C2 + 1:C2 + 2],
                                         start=True, stop=True)
                        nc.vector.tensor_tensor(
                            out=alp3[:, k, :], in0=vg3[:, k, C2:C2 + 1],
                            in1=adp[:], op=OP.add)

                    asc = pdmd.tile([P, st + 1], F32, tag="asc2")
                    nc.vector.tensor_scalar(out=asc[:], in0=alp[:],
                                            scalar1=cfg.NEG, scalar2=None,
                                            op0=OP.mult)
                    lrl = pdmd.tile([P, st + 1], F32, tag="lrl2")
                    nc.vector.tensor_tensor(out=lrl[:], in0=alp[:], in1=asc[:],
                                            op=OP.max)
                    eeh = pdmd.tile([P, st + 1], F16, tag="eeh2")
                    nc.scalar.activation(out=eeh[:], in_=lrl[:], func=AF.Exp)

                    xx = pdig.tile([P, (st + 1) * (C2 + 1)], F16, tag="xx2")
                    xx3 = xx[:].rearrange("p (k c) -> p k c", c=C2 + 1)
                    nc.vector.tensor_copy(
                        out=xx3[:, :, C2:C2 + 1],
                        in_=eeh[:].rearrange("p (k o) -> p k o", o=1))
                    nc.vector.tensor_tensor(
                        out=xx3[:, 0:st, 0:C2],
                        in0=vg3[:, :, 0:C2],
                        in1=eeh[:, 0:st].rearrange("p (k o) -> p k o", o=1)
                            .to_broadcast([P, st, C2]),
                        op=OP.mult)
                    nc.vector.tensor_tensor(
                        out=xx3[:, st, 0:C2],
                        in0=g2f[:, 0:C2],
                        in1=eeh[:, st:st + 1].to_broadcast([P, C2]),
                        op=OP.mult)

                    ps = pdp.tile([P, C2 + 1], F32, space="PSUM", tag="ps2")
                    for k in range(st + 1):
                        if k in ov_ids:
                            j = ov_ids.index(k)
                            lhsT = mk[:, j * P:(j + 1) * P]
                        else:
                            lhsT = idsb[:]
                        nc.tensor.matmul(ps[:], lhsT=lhsT, rhs=xx3[:, k, :],
                                         start=(k == 0), stop=(k == st))
                    rec = pdmd.tile([P, 1], F32, tag="rec2")
                    nc.vector.reciprocal(out=rec[:ndst, :],
                                         in_=ps[:ndst, C2:C2 + 1])
                    o2 = pdmd.tile([P, C2], F32, tag="o2")
                    nc.vector.tensor_scalar(out=o2[:ndst, :],
                                            in0=ps[:ndst, 0:C2],
                                            scalar1=rec[:ndst, :],
                                            scalar2=None, op0=OP.mult)
                    nc.sync.dma_start(out=PRE[t * P:t * P + ndst, :],
                                      in_=o2[:ndst, :])

                    olo += cl
                    ohi += ch
                    omk += nov

            # ---- phase E: one sigmoid sweep -------------------------------
            with tc.tile_pool(name="pe", bufs=2) as pe:
                FW = NT * C2
                pre_f = PRE.rearrange("(a b) c -> a (b c)", a=P)
                out_f = out.rearrange("(a b) c -> a (b c)", a=P)
                pei = pe.tile([P, FW], F32, tag="pei")
                nc.sync.dma_start(out=pei[:], in_=pre_f)
                peo = pe.tile([P, FW], F32, tag="peo")
                nc.scalar.activation(out=peo[:], in_=pei[:], func=AF.Sigmoid)
                nc.sync.dma_start(out=out_f, in_=peo[:])

    nc.compile()
    return nc


_CACHE: dict = {}


def _get_module(cfg, plan):
    key = (cfg.N, cfg.E, plan.key)
    if key not in _CACHE:
        _CACHE[key] = _build(cfg, plan)
    return _CACHE[key]


def _run(cfg, inputs, trace=False):
    plan, common, per_core = _prep_host(
        cfg, inputs["x"], inputs["edge_index"], inputs["W1"],
        inputs["a_src1"], inputs["a_dst1"], inputs["b1"], inputs["W2"],
        inputs["a_src2"], inputs["a_dst2"], inputs["b2"])
    nc = _get_module(cfg, plan)
    in_maps = [dict(common, **pc) for pc in per_core]
    res = run_bass_kernel_spmd(nc, in_maps, core_ids=list(range(cfg.NC)),
                               trace=trace)
    shards = [np.asarray(res.results[c]["out"])[:cfg.NPC]
              for c in range(cfg.NC)]
    full = np.concatenate(shards, axis=0).astype(np.float32)
    return (full, res) if trace else full


def kernel(**inputs) -> np.ndarray:
    cfg = GATConfig(n=50000, e=800000)
    return _run(cfg, inputs)
